# revision 24
# baseline (speedup 1.0000x reference)
"""Trainium2 Bass kernel for nn_CustomLlamaModel (2-layer MQA llama, B=1 S=2048
H=1024 HQ=16 HKV=1 FF=4096), tensor-parallel over 8 NeuronCores.

Strategy (per sharding hint): column-parallel q/gate/up, row-parallel o/down,
KV head replicated; residual stream token-sharded (256 tokens/core) and kept
feature-major [H, tok] in SBUF.  Per layer: rmsnorm (PE ones-matmul partition
reduce) -> AllGather(h) -> qkv+rope -> transposed-scores attention (softmax
along partitions via exp + ones-column folded into the attn@v matmul) ->
row-parallel o-proj -> ReduceScatter -> residual add -> rmsnorm -> AllGather
-> gate/up/silu -> row-parallel down -> ReduceScatter -> residual add.
All matmuls run in float32r (full PE rate at N>=256, ~1e-4 matmul rel-err).
ln1/ln2 and the 1/sqrt(D) attention scale are folded into weights host-side;
the embedding gather runs host-side (pure numpy indexing).
"""
import sys

sys.path.insert(0, "/opt/trn_rl_repo")

import numpy as np
import orjson

import concourse.bass as bass
import concourse.mybir as mybir
import concourse.tile as tile
from concourse import bass_utils
from concourse.masks import make_identity

# ---------------------------------------------------------------------------
# Walrus in this container supports only ONE sync-wait per instruction, but
# Tile's scheduler emits multi-wait instructions.  Post-process the BIR JSON:
# split each multi-wait instruction into single-wait NoOps (same engine,
# program-order before the original).
# ---------------------------------------------------------------------------
_orig_to_json_bytes = bass.Bass.to_json_bytes
_MW = [0]


def _split_multiwait(d):
    changed = False

    def fix_block(bb):
        nonlocal changed
        insts = bb.get("instructions")
        if not insts:
            return
        out = []
        for ins in insts:
            si = ins.get("sync_info")
            if si:
                ow = si.get("on_wait") or []
                if len(ow) > 1:
                    changed = True
                    for w in ow[:-1]:
                        _MW[0] += 1
                        out.append({
                            "debug": ins.get("debug", 0),
                            "engine": ins["engine"],
                            "ins": [],
                            "outs": [],
                            "name": f"{ins['name']}-mw{_MW[0]}",
                            "opcode": "NoOp",
                            "sync_info": {"on_update": [], "on_wait": [w]},
                        })
                    si["on_wait"] = [ow[-1]]
            out.append(ins)
        bb["instructions"] = out

    def rec(o):
        if isinstance(o, dict):
            if isinstance(o.get("instructions"), list):
                fix_block(o)
            for v in o.values():
                rec(v)
        elif isinstance(o, list):
            for v in o:
                rec(v)

    for fn in d.get("functions", []):
        rec(fn)
    return changed


def _patched_to_json_bytes(self):
    raw = _orig_to_json_bytes(self)
    d = orjson.loads(raw)
    if _split_multiwait(d):
        return orjson.dumps(d)
    return raw


bass.Bass.to_json_bytes = _patched_to_json_bytes

# ---------------------------------------------------------------------------
# Model / sharding constants
# ---------------------------------------------------------------------------
S, H, D, HQ, FF, L, V = 2048, 1024, 64, 16, 4096, 2, 32000
EPS = 1e-6
NCORES = 8
TOK = S // NCORES       # residual tokens per core (256)
QH = HQ // NCORES       # heads per core (2)
QD = QH * D             # q dims per core (128)
FFL = FF // NCORES      # ff dims per core (512)
NCH = 4                 # token chunks of 512
CH = S // NCH           # 512
KT = S // 128           # 16 key-token tiles
HT = H // 128           # 8 hidden feature tiles
F32 = mybir.dt.float32
F32R = mybir.dt.float32r
MUL = mybir.AluOpType.mult
ADD = mybir.AluOpType.add
RG = [list(range(NCORES))]

_CACHED_NC = None


def _build_nc():
    nc = bass.Bass()
    x0 = nc.dram_tensor("x0", [H, TOK], F32, kind="ExternalInput")
    cosf = nc.dram_tensor("cosf", [64, S], F32, kind="ExternalInput")
    sinf = nc.dram_tensor("sinf", [64, S], F32, kind="ExternalInput")
    W = []
    for l in range(L):
        W.append({
            "wq": nc.dram_tensor(f"wq{l}", [H, QD], F32R, kind="ExternalInput"),
            "wk": nc.dram_tensor(f"wk{l}", [H, D], F32R, kind="ExternalInput"),
            "wv": nc.dram_tensor(f"wv{l}", [H, D], F32R, kind="ExternalInput"),
            "wo": nc.dram_tensor(f"wo{l}", [QD, H], F32R, kind="ExternalInput"),
            "wg": nc.dram_tensor(f"wg{l}", [H, FFL], F32R, kind="ExternalInput"),
            "wu": nc.dram_tensor(f"wu{l}", [H, FFL], F32R, kind="ExternalInput"),
            "wd": nc.dram_tensor(f"wd{l}", [FFL, H], F32R, kind="ExternalInput"),
        })
    xout = nc.dram_tensor("xout", [H, TOK], F32, kind="ExternalOutput")

    with tile.TileContext(nc) as tc:
        with (
            tc.tile_pool(name="const", bufs=1) as pconst,
            tc.tile_pool(name="resid", bufs=1) as presid,
            tc.tile_pool(name="wts", bufs=1) as pwts,
            tc.tile_pool(name="stream", bufs=3) as pstream,
            tc.tile_pool(name="acts", bufs=1) as pacts,
            tc.tile_pool(name="big", bufs=1) as pbig,
            tc.tile_pool(name="small", bufs=2) as psmall,
            tc.tile_pool(name="exp", bufs=2) as pexp,
            tc.tile_pool(name="outw", bufs=3) as poutw,
            tc.tile_pool(name="dram", bufs=2, space="DRAM") as pdram,
        ):
            # constants
            identf = pconst.tile([128, 128], F32, tag="identf")
            make_identity(nc, identf[:])
            ident = pconst.tile([128, 128], F32R, tag="ident")
            nc.vector.tensor_copy(ident[:], identf[:])
            onesf = pconst.tile([128, 128], F32, tag="onesf")
            nc.vector.memset(onesf[:], 1.0)
            ones = pconst.tile([128, 128], F32R, tag="ones")
            nc.vector.tensor_copy(ones[:], onesf[:])
            epst = pconst.tile([128, 1], F32, tag="eps")
            nc.gpsimd.memset(epst[:], EPS)
            cos_sb = pconst.tile([64, S], F32, tag="cos")
            sin_sb = pconst.tile([64, S], F32, tag="sin")
            nc.sync.dma_start(cos_sb[:], cosf[:])
            nc.sync.dma_start(sin_sb[:], sinf[:])

            # residual x, feature-major: block ht -> cols [ht*TOK:(ht+1)*TOK]
            x_sb = presid.tile([128, HT * TOK], F32, tag="x")
            for ht in range(HT):
                nc.sync.dma_start(
                    x_sb[:, ht * TOK:(ht + 1) * TOK],
                    x0[ht * 128:(ht + 1) * 128, :],
                )

            def rmsnorm_ag(tag):
                """x_sb -> normalized h (f32r) -> bounce -> AllGather.
                Returns the AG output dram tile [8*H, TOK]."""
                with tc.tile_pool(name=f"ps_n_{tag}", bufs=2,
                                  space="PSUM") as pps:
                    x2 = pbig.tile([128, HT * TOK], F32R, tag="rot")
                    nc.vector.tensor_tensor(x2[:], x_sb[:], x_sb[:], op=MUL)
                    ssq = pps.tile([1, TOK], F32, tag="ssq")
                    for ht in range(HT):
                        nc.tensor.matmul(
                            ssq[:],
                            ones[:, 0:1],
                            x2[:, ht * TOK:(ht + 1) * TOK],
                            start=(ht == 0),
                            stop=(ht == HT - 1),
                        )
                    sstd = psmall.tile([1, TOK], F32, tag="sstd")
                    nc.scalar.activation(
                        sstd[:], ssq[:], mybir.ActivationFunctionType.Sqrt,
                        bias=epst[0:1, :], scale=1.0 / H,
                    )
                    rinv = psmall.tile([1, TOK], F32R, tag="rinv")
                    with nc.allow_low_precision(reason="f32r is fp32 bits"):
                        nc.vector.reciprocal(rinv[:], sstd[:])
                    rb = pps.tile([128, TOK], F32, tag="rb")
                    nc.tensor.matmul(rb[:], ones[0:1, :], rinv[:],
                                     start=True, stop=True)
                    h_sb = pbig.tile([128, HT * TOK], F32R, tag="h")
                    for ht in range(HT):
                        nc.vector.tensor_tensor(
                            h_sb[:, ht * TOK:(ht + 1) * TOK],
                            x_sb[:, ht * TOK:(ht + 1) * TOK],
                            rb[:],
                            op=MUL,
                        )
                    ag_in = pdram.tile([H, TOK], F32R, tag="ag_in")
                    for ht in range(HT):
                        nc.sync.dma_start(
                            ag_in[ht * 128:(ht + 1) * 128, :],
                            h_sb[:, ht * TOK:(ht + 1) * TOK],
                        )
                    ag_out = pdram.tile([NCORES * H, TOK], F32R, tag="ag_out",
                                        addr_space="Shared")
                    nc.gpsimd.collective_compute(
                        "AllGather",
                        mybir.AluOpType.bypass,
                        replica_groups=RG,
                        ins=[ag_in[:].opt()],
                        outs=[ag_out[:].opt()],
                    )
                return ag_out

            def load_rhs(ag_out, k, n):
                """Stream h^T chunk [128, CH] (features k*128.., tokens n*CH..)
                from the AllGather result."""
                rhs = pstream.tile([128, CH], F32R, tag="rhs")
                for j in range(CH // TOK):
                    s = (n * CH) // TOK + j
                    nc.sync.dma_start(
                        rhs[:, j * TOK:(j + 1) * TOK],
                        ag_out[s * H + k * 128: s * H + (k + 1) * 128, :],
                    )
                return rhs

            def reduce_scatter(o_parts):
                """o_parts: callable(m, n) -> sbuf tile [128, CH] holding the
                partial output for features m*128.., tokens n*CH..  Writes the
                s-blocked bounce, runs RS, adds the result into x_sb."""
                rs_in = pdram.tile([NCORES * H, TOK], F32, tag="rs_in")
                for n in range(NCH):
                    for m in range(HT):
                        pt = o_parts(m, n)
                        for j in range(CH // TOK):
                            s = (n * CH) // TOK + j
                            nc.sync.dma_start(
                                rs_in[s * H + m * 128: s * H + (m + 1) * 128, :],
                                pt[:, j * TOK:(j + 1) * TOK],
                            )
                rs_out = pdram.tile([H, TOK], F32, tag="rs_out")
                nc.gpsimd.collective_compute(
                    "ReduceScatter",
                    mybir.AluOpType.add,
                    replica_groups=RG,
                    ins=[rs_in[:].opt()],
                    outs=[rs_out[:].opt()],
                )
                for ht in range(HT):
                    radd = poutw.tile([128, TOK], F32, tag="radd")
                    nc.sync.dma_start(radd[:],
                                      rs_out[ht * 128:(ht + 1) * 128, :])
                    nc.vector.tensor_tensor(
                        x_sb[:, ht * TOK:(ht + 1) * TOK],
                        x_sb[:, ht * TOK:(ht + 1) * TOK],
                        radd[:],
                        op=ADD,
                    )

            def rope(src, c0):
                """In-place RoPE on an S-column window of src starting at
                column c0 (feature-major [64, .] head block; cos/sin tables
                carry the sign pattern)."""
                cs = slice(c0, c0 + S)
                rot = pbig.tile([128, HT * TOK], F32R, tag="rot")
                nc.vector.tensor_copy(rot[0:32, 0:S], src[32:64, cs])
                nc.vector.tensor_copy(rot[32:64, 0:S], src[0:32, cs])
                nc.vector.tensor_tensor(
                    rot[0:64, 0:S], rot[0:64, 0:S], sin_sb[:, 0:S], op=MUL,
                )
                nc.vector.tensor_tensor(
                    src[:, cs], src[:, cs], cos_sb[:, 0:S], op=MUL,
                )
                nc.vector.tensor_tensor(
                    src[:, cs], src[:, cs], rot[0:64, 0:S], op=ADD,
                )

            for l in range(L):
                w = W[l]
                # ---------------- attention ----------------
                ag1 = rmsnorm_ag(f"a{l}")

                # weight loads (lhsT layouts)
                wq_sb = pwts.tile([128, HT * QD], F32R, tag="wq")
                wk_sb = pwts.tile([128, HT * D], F32R, tag="wk")
                wv_sb = pwts.tile([128, HT * D], F32R, tag="wv")
                wo_sb = pwts.tile([128, HT * 128], F32R, tag="wo")
                for k in range(HT):
                    nc.sync.dma_start(
                        wq_sb[:, k * QD:(k + 1) * QD],
                        w["wq"][k * 128:(k + 1) * 128, :],
                    )
                    nc.sync.dma_start(
                        wk_sb[:, k * D:(k + 1) * D],
                        w["wk"][k * 128:(k + 1) * 128, :],
                    )
                    nc.sync.dma_start(
                        wv_sb[:, k * D:(k + 1) * D],
                        w["wv"][k * 128:(k + 1) * 128, :],
                    )
                    nc.sync.dma_start(
                        wo_sb[:, k * 128:(k + 1) * 128],
                        w["wo"][:, k * 128:(k + 1) * 128],
                    )

                # q head-blocked [64, 2S]: head hh lives at cols hh*S..
                q_hb = pbig.tile([64, 2 * S], F32R, tag="q_hb")
                kTt = pbig.tile([64, S], F32R, tag="kT")
                vT = pbig.tile([64, S], F32R, tag="vT")
                with tc.tile_pool(name=f"ps_qkv{l}", bufs=2,
                                  space="PSUM") as pps:
                    for n in range(NCH):
                        pq = pps.tile([128, CH], F32, tag="pq")
                        pk = pps.tile([64, CH], F32, tag="pk")
                        pv = pps.tile([64, CH], F32, tag="pv")
                        for k in range(HT):
                            rhs = load_rhs(ag1, k, n)
                            nc.tensor.matmul(
                                pq[:], wq_sb[:, k * QD:(k + 1) * QD], rhs[:],
                                start=(k == 0), stop=(k == HT - 1),
                            )
                            nc.tensor.matmul(
                                pk[:], wk_sb[:, k * D:(k + 1) * D], rhs[:],
                                start=(k == 0), stop=(k == HT - 1),
                            )
                            nc.tensor.matmul(
                                pv[:], wv_sb[:, k * D:(k + 1) * D], rhs[:],
                                start=(k == 0), stop=(k == HT - 1),
                            )
                        ns = slice(n * CH, (n + 1) * CH)
                        nc.vector.tensor_copy(q_hb[:, ns], pq[0:64, :])
                        nc.vector.tensor_copy(
                            q_hb[:, S + n * CH:S + (n + 1) * CH],
                            pq[64:128, :],
                        )
                        nc.vector.tensor_copy(kTt[:, ns], pk[:])
                        nc.vector.tensor_copy(vT[:, ns], pv[:])

                    rope(q_hb, 0)
                    rope(q_hb, S)
                    rope(kTt, 0)

                    # token-major v with ones column (softmax sum rides attn@v)
                    v_tok = pbig.tile([128, KT * 65], F32R, tag="v_tok")
                    for kt in range(KT):
                        nc.vector.tensor_copy(
                            v_tok[:, kt * 65 + 64:kt * 65 + 65],
                            onesf[:, 0:1],
                        )
                    for kt in range(KT):
                        pvt = pps.tile([128, 64], F32R, tag="pvt")
                        nc.tensor.transpose(
                            pvt[:], vT[:, kt * 128:(kt + 1) * 128],
                            ident[0:64, 0:64],
                        )
                        nc.vector.tensor_copy(
                            v_tok[:, kt * 65:kt * 65 + 64], pvt[:]
                        )

                oT = pbig.tile([128, S], F32R, tag="oT")
                with tc.tile_pool(name=f"ps_att{l}", bufs=1,
                                  space="PSUM") as ppa:
                    for hh in range(QH):
                        pav = ppa.tile([65, S], F32, tag="pav")
                        for kt in range(KT):
                            for half in range(2):
                                psc = ppa.tile([128, 2 * CH], F32, tag="psc",
                                               bufs=2)
                                for j in range(2):
                                    n = half * 2 + j
                                    nc.tensor.matmul(
                                        psc[:, j * CH:(j + 1) * CH],
                                        kTt[:, kt * 128:(kt + 1) * 128],
                                        q_hb[:, hh * S + n * CH:
                                             hh * S + (n + 1) * CH],
                                        start=True, stop=True,
                                    )
                                et = pexp.tile([128, 2 * CH], F32R, tag="et")
                                nc.scalar.activation(
                                    et[:], psc[:],
                                    mybir.ActivationFunctionType.Exp,
                                )
                                for j in range(2):
                                    n = half * 2 + j
                                    nc.tensor.matmul(
                                        pav[:, n * CH:(n + 1) * CH],
                                        v_tok[:, kt * 65:(kt + 1) * 65],
                                        et[:, j * CH:(j + 1) * CH],
                                        start=(kt == 0), stop=(kt == KT - 1),
                                        skip_group_check=True,
                                    )
                        for n in range(NCH):
                            ns = slice(n * CH, (n + 1) * CH)
                            rec = psmall.tile([1, CH], F32R, tag="rec")
                            with nc.allow_low_precision(
                                reason="f32r is fp32 bits"
                            ):
                                nc.vector.reciprocal(rec[:], pav[64:65, ns])
                            rbc = ppa.tile([64, CH], F32, tag="psc", bufs=2)
                            nc.tensor.matmul(
                                rbc[:], ones[0:1, 0:64], rec[:],
                                start=True, stop=True,
                            )
                            rbs = poutw.tile([64, CH], F32, tag="ut")
                            nc.vector.tensor_copy(rbs[:], rbc[:])
                            nc.vector.tensor_tensor(
                                oT[hh * 64:(hh + 1) * 64, ns],
                                pav[0:64, ns], rbs[:], op=MUL,
                            )

                # row-parallel o-proj -> RS -> residual add
                with tc.tile_pool(name=f"ps_o{l}", bufs=3,
                                  space="PSUM") as ppo:
                    def o_part(m, n, _oT=oT, _wo=wo_sb, _pp=ppo):
                        po = _pp.tile([128, CH], F32, tag="po")
                        nc.tensor.matmul(
                            po[:], _wo[:, m * 128:(m + 1) * 128],
                            _oT[:, n * CH:(n + 1) * CH],
                            start=True, stop=True,
                        )
                        ot = poutw.tile([128, CH], F32, tag="opart")
                        nc.vector.tensor_copy(ot[:], po[:])
                        return ot

                    reduce_scatter(o_part)

                # ---------------- mlp ----------------
                ag2 = rmsnorm_ag(f"m{l}")
                act = pacts.tile([128, 4 * S], F32R, tag="act")
                with tc.tile_pool(name=f"ps_gu{l}", bufs=2,
                                  space="PSUM") as ppg:
                    for f in range(4):
                        wg_sb = pwts.tile([128, HT * 128], F32R, tag="wg",
                                          bufs=2)
                        wu_sb = pwts.tile([128, HT * 128], F32R, tag="wu",
                                          bufs=2)
                        for k in range(HT):
                            nc.sync.dma_start(
                                wg_sb[:, k * 128:(k + 1) * 128],
                                w["wg"][k * 128:(k + 1) * 128,
                                        f * 128:(f + 1) * 128],
                            )
                            nc.sync.dma_start(
                                wu_sb[:, k * 128:(k + 1) * 128],
                                w["wu"][k * 128:(k + 1) * 128,
                                        f * 128:(f + 1) * 128],
                            )
                        for n in range(NCH):
                            pg = ppg.tile([128, CH], F32, tag="pg")
                            pu = ppg.tile([128, CH], F32, tag="pu")
                            for k in range(HT):
                                rhs = load_rhs(ag2, k, n)
                                nc.tensor.matmul(
                                    pg[:], wg_sb[:, k * 128:(k + 1) * 128],
                                    rhs[:],
                                    start=(k == 0), stop=(k == HT - 1),
                                )
                                nc.tensor.matmul(
                                    pu[:], wu_sb[:, k * 128:(k + 1) * 128],
                                    rhs[:],
                                    start=(k == 0), stop=(k == HT - 1),
                                )
                            ns = slice(f * S + n * CH, f * S + (n + 1) * CH)
                            nc.scalar.activation(
                                act[:, ns], pg[:],
                                mybir.ActivationFunctionType.Silu,
                            )
                            ut = poutw.tile([128, CH], F32R, tag="ut")
                            nc.vector.tensor_copy(ut[:], pu[:])
                            nc.vector.tensor_tensor(
                                act[:, ns], act[:, ns], ut[:], op=MUL,
                            )

                wd_sb = pwts.tile([128, 4 * H], F32R, tag="wd")
                for fk in range(4):
                    nc.sync.dma_start(
                        wd_sb[:, fk * H:(fk + 1) * H],
                        w["wd"][fk * 128:(fk + 1) * 128, :],
                    )

                with tc.tile_pool(name=f"ps_d{l}", bufs=3,
                                  space="PSUM") as ppd:
                    def d_part(m, n, _act=act, _wd=wd_sb, _pp=ppd):
                        pd = _pp.tile([128, CH], F32, tag="pd")
                        for fk in range(4):
                            nc.tensor.matmul(
                                pd[:],
                                _wd[:, fk * H + m * 128:
                                    fk * H + (m + 1) * 128],
                                _act[:, fk * S + n * CH:
                                     fk * S + (n + 1) * CH],
                                start=(fk == 0), stop=(fk == 3),
                            )
                        ot = poutw.tile([128, CH], F32, tag="opart")
                        nc.vector.tensor_copy(ot[:], pd[:])
                        return ot

                    reduce_scatter(d_part)

            # final output
            for ht in range(HT):
                nc.sync.dma_start(
                    xout[ht * 128:(ht + 1) * 128, :],
                    x_sb[:, ht * TOK:(ht + 1) * TOK],
                )
    return nc


def _get_nc():
    global _CACHED_NC
    if _CACHED_NC is None:
        _CACHED_NC = _build_nc()
    return _CACHED_NC


def _host_prep(inputs):
    """Fold ln/scale into weights, pre-transpose shards, embed gather, rope
    tables.  Returns in_maps (list of dicts, one per core)."""
    ids = np.asarray(inputs["input_ids"])[0]          # [S] int32
    embed = np.asarray(inputs["embed"], np.float32)   # [V, H]
    x = embed[ids]                                    # [S, H]

    inv = 1.0 / (10000.0 ** (np.arange(0, D, 2, dtype=np.float32) / D))  # [32]
    freqs = np.arange(S, dtype=np.float32)[:, None] * inv[None, :]       # [S,32]
    cosT = np.cos(freqs).T.astype(np.float32)   # [32, S]
    sinT = np.sin(freqs).T.astype(np.float32)
    cosF = np.ascontiguousarray(np.tile(cosT, (2, 1)))            # [64, S]
    sinF = np.ascontiguousarray(np.concatenate([-sinT, sinT], 0))  # [64, S]

    scale = np.float32(1.0 / np.sqrt(D))
    in_maps = []
    for c in range(NCORES):
        m = {
            "x0": np.ascontiguousarray(x[c * TOK:(c + 1) * TOK, :].T),
            "cosf": cosF,
            "sinf": sinF,
        }
        for l in range(L):
            ln1 = np.asarray(inputs["ln1"], np.float32)[l]
            ln2 = np.asarray(inputs["ln2"], np.float32)[l]
            wq = np.asarray(inputs["Wq"], np.float32)[l] * ln1[None, :] * scale
            wk = np.asarray(inputs["Wk"], np.float32)[l] * ln1[None, :]
            wv = np.asarray(inputs["Wv"], np.float32)[l] * ln1[None, :]
            wo = np.asarray(inputs["Wo"], np.float32)[l]
            wg = np.asarray(inputs["Wg"], np.float32)[l] * ln2[None, :]
            wu = np.asarray(inputs["Wu"], np.float32)[l] * ln2[None, :]
            wd = np.asarray(inputs["Wd"], np.float32)[l]
            m[f"wq{l}"] = np.ascontiguousarray(wq[c * QD:(c + 1) * QD, :].T)
            m[f"wk{l}"] = np.ascontiguousarray(wk.T)
            m[f"wv{l}"] = np.ascontiguousarray(wv.T)
            m[f"wo{l}"] = np.ascontiguousarray(wo[:, c * QD:(c + 1) * QD].T)
            m[f"wg{l}"] = np.ascontiguousarray(wg[c * FFL:(c + 1) * FFL, :].T)
            m[f"wu{l}"] = np.ascontiguousarray(wu[c * FFL:(c + 1) * FFL, :].T)
            m[f"wd{l}"] = np.ascontiguousarray(wd[:, c * FFL:(c + 1) * FFL].T)
        in_maps.append(m)
    return in_maps


def kernel(**inputs) -> np.ndarray:
    nc = _get_nc()
    in_maps = _host_prep(inputs)
    res = bass_utils.run_bass_kernel_spmd(
        nc, in_maps, core_ids=list(range(NCORES))
    )
    out = np.empty((1, S, H), np.float32)
    for c in range(NCORES):
        out[0, c * TOK:(c + 1) * TOK, :] = res.results[c]["xout"].T
    return out


# revision 26
# speedup vs baseline: 1.3870x; 1.3870x over previous
"""Trainium2 Bass kernel for nn_CustomLlamaModel (2-layer MQA llama, B=1 S=2048
H=1024 HQ=16 HKV=1 FF=4096), tensor-parallel over 8 NeuronCores.

Strategy (per sharding hint): column-parallel q/gate/up, row-parallel o/down,
KV head replicated; residual stream token-sharded (256 tokens/core) and kept
feature-major [H, tok] fp32 in SBUF.  Per layer: rmsnorm (PE ones-matmul
partition reduce, fp32r) -> AllGather(h, bf16) -> qkv+rope -> transposed-
scores attention (softmax along partitions via exp + ones-column folded into
the attn@v matmul) -> row-parallel o-proj -> ReduceScatter(bf16) -> residual
add -> rmsnorm -> AllGather -> gate/up/silu -> row-parallel down ->
ReduceScatter -> residual add.  Matmul pipeline runs in bf16 (fp32 PSUM
accumulate); the gathered h stays fully SBUF-resident.  ln1/ln2 and the
1/sqrt(D) attention scale are folded into weights host-side; the embedding
gather runs host-side (pure numpy indexing).
"""
import sys

sys.path.insert(0, "/opt/trn_rl_repo")

import ml_dtypes
import numpy as np
import orjson

import concourse.bass as bass
import concourse.mybir as mybir
import concourse.tile as tile
from concourse import bass_utils
from concourse.masks import make_identity

# ---------------------------------------------------------------------------
# Walrus in this container supports only ONE sync-wait per instruction, but
# Tile's scheduler emits multi-wait instructions.  Post-process the BIR JSON:
# split each multi-wait instruction into single-wait NoOps (same engine,
# program-order before the original).
# ---------------------------------------------------------------------------
_orig_to_json_bytes = bass.Bass.to_json_bytes
_MW = [0]


def _split_multiwait(d):
    changed = False

    def fix_block(bb):
        nonlocal changed
        insts = bb.get("instructions")
        if not insts:
            return
        out = []
        for ins in insts:
            si = ins.get("sync_info")
            if si:
                ow = si.get("on_wait") or []
                if len(ow) > 1:
                    changed = True
                    for w in ow[:-1]:
                        _MW[0] += 1
                        out.append({
                            "debug": ins.get("debug", 0),
                            "engine": ins["engine"],
                            "ins": [],
                            "outs": [],
                            "name": f"{ins['name']}-mw{_MW[0]}",
                            "opcode": "NoOp",
                            "sync_info": {"on_update": [], "on_wait": [w]},
                        })
                    si["on_wait"] = [ow[-1]]
            out.append(ins)
        bb["instructions"] = out

    def rec(o):
        if isinstance(o, dict):
            if isinstance(o.get("instructions"), list):
                fix_block(o)
            for v in o.values():
                rec(v)
        elif isinstance(o, list):
            for v in o:
                rec(v)

    for fn in d.get("functions", []):
        rec(fn)
    return changed


def _patched_to_json_bytes(self):
    raw = _orig_to_json_bytes(self)
    d = orjson.loads(raw)
    if _split_multiwait(d):
        return orjson.dumps(d)
    return raw


bass.Bass.to_json_bytes = _patched_to_json_bytes

# ---------------------------------------------------------------------------
# Model / sharding constants
# ---------------------------------------------------------------------------
S, H, D, HQ, FF, L, V = 2048, 1024, 64, 16, 4096, 2, 32000
EPS = 1e-6
NCORES = 8
TOK = S // NCORES       # residual tokens per core (256)
QH = HQ // NCORES       # heads per core (2)
QD = QH * D             # q dims per core (128)
FFL = FF // NCORES      # ff dims per core (512)
NCH = 4                 # token chunks of 512
CH = S // NCH           # 512
KT = S // 128           # 16 key-token tiles
HT = H // 128           # 8 hidden feature tiles
F32 = mybir.dt.float32
F32R = mybir.dt.float32r
BF16 = mybir.dt.bfloat16
MUL = mybir.AluOpType.mult
ADD = mybir.AluOpType.add
RG = [list(range(NCORES))]
BF = ml_dtypes.bfloat16

_CACHED_NC = None


def _build_nc():
    nc = bass.Bass()
    x0 = nc.dram_tensor("x0", [H, TOK], F32, kind="ExternalInput")
    cosf = nc.dram_tensor("cosf", [64, S], BF16, kind="ExternalInput")
    sinf = nc.dram_tensor("sinf", [64, S], BF16, kind="ExternalInput")
    W = []
    for l in range(L):
        W.append({
            "wq": nc.dram_tensor(f"wq{l}", [H, QD], BF16, kind="ExternalInput"),
            "wk": nc.dram_tensor(f"wk{l}", [H, D], BF16, kind="ExternalInput"),
            "wv": nc.dram_tensor(f"wv{l}", [H, D], BF16, kind="ExternalInput"),
            "wo": nc.dram_tensor(f"wo{l}", [QD, H], BF16, kind="ExternalInput"),
            "wg": nc.dram_tensor(f"wg{l}", [H, FFL], BF16, kind="ExternalInput"),
            "wu": nc.dram_tensor(f"wu{l}", [H, FFL], BF16, kind="ExternalInput"),
            "wd": nc.dram_tensor(f"wd{l}", [FFL, H], BF16, kind="ExternalInput"),
        })
    xout = nc.dram_tensor("xout", [H, TOK], F32, kind="ExternalOutput")

    with tile.TileContext(nc) as tc:
        with (
            tc.tile_pool(name="const", bufs=1) as pconst,
            tc.tile_pool(name="resid", bufs=1) as presid,
            tc.tile_pool(name="wts", bufs=1) as pwts,
            tc.tile_pool(name="hfull", bufs=1) as phf,
            tc.tile_pool(name="acts", bufs=1) as pacts,
            tc.tile_pool(name="big", bufs=1) as pbig,
            tc.tile_pool(name="small", bufs=2) as psmall,
            tc.tile_pool(name="exp", bufs=2) as pexp,
            tc.tile_pool(name="outw", bufs=3) as poutw,
            tc.tile_pool(name="dram", bufs=2, space="DRAM") as pdram,
        ):
            # constants
            identf = pconst.tile([128, 128], F32, tag="identf")
            make_identity(nc, identf[:])
            ident = pconst.tile([128, 128], BF16, tag="ident")
            nc.vector.tensor_copy(ident[:], identf[:])
            onesf = pconst.tile([128, 128], F32, tag="onesf")
            nc.vector.memset(onesf[:], 1.0)
            ones = pconst.tile([128, 128], BF16, tag="ones")
            nc.vector.tensor_copy(ones[:], onesf[:])
            onesr = pconst.tile([128, 128], F32R, tag="onesr")
            nc.vector.tensor_copy(onesr[:], onesf[:])
            epst = pconst.tile([128, 1], F32, tag="eps")
            nc.gpsimd.memset(epst[:], EPS)
            cos_sb = pconst.tile([64, S], BF16, tag="cos")
            sin_sb = pconst.tile([64, S], BF16, tag="sin")
            nc.sync.dma_start(cos_sb[:], cosf[:])
            nc.sync.dma_start(sin_sb[:], sinf[:])

            # residual x, feature-major: block ht -> cols [ht*TOK:(ht+1)*TOK]
            x_sb = presid.tile([128, HT * TOK], F32, tag="x")
            for ht in range(HT):
                nc.sync.dma_start(
                    x_sb[:, ht * TOK:(ht + 1) * TOK],
                    x0[ht * 128:(ht + 1) * 128, :],
                )

            def rmsnorm_ag_load(tag):
                """x_sb -> normalized h (bf16) -> bounce -> AllGather -> load
                into a fully-resident SBUF tile h_full [128, HT*S] (block k =
                features k*128.., cols = tokens in order)."""
                with tc.tile_pool(name=f"ps_n_{tag}", bufs=2,
                                  space="PSUM") as pps:
                    x2 = pbig.tile([128, HT * TOK], F32R, tag="rot")
                    nc.vector.tensor_tensor(x2[:], x_sb[:], x_sb[:], op=MUL)
                    ssq = pps.tile([1, TOK], F32, tag="ssq")
                    for ht in range(HT):
                        nc.tensor.matmul(
                            ssq[:],
                            onesr[:, 0:1],
                            x2[:, ht * TOK:(ht + 1) * TOK],
                            start=(ht == 0),
                            stop=(ht == HT - 1),
                        )
                    sstd = psmall.tile([1, TOK], F32, tag="sstd")
                    nc.scalar.activation(
                        sstd[:], ssq[:], mybir.ActivationFunctionType.Sqrt,
                        bias=epst[0:1, :], scale=1.0 / H,
                    )
                    rinv = psmall.tile([1, TOK], F32R, tag="rinv")
                    with nc.allow_low_precision(reason="f32r is fp32 bits"):
                        nc.vector.reciprocal(rinv[:], sstd[:])
                    rb = pps.tile([128, TOK], F32, tag="rb")
                    nc.tensor.matmul(rb[:], onesr[0:1, :], rinv[:],
                                     start=True, stop=True)
                    h_sb = pbig.tile([128, HT * TOK], BF16, tag="h")
                    for ht in range(HT):
                        nc.vector.tensor_tensor(
                            h_sb[:, ht * TOK:(ht + 1) * TOK],
                            x_sb[:, ht * TOK:(ht + 1) * TOK],
                            rb[:],
                            op=MUL,
                        )
                    ag_in = pdram.tile([H, TOK], BF16, tag="ag_in")
                    for ht in range(HT):
                        nc.sync.dma_start(
                            ag_in[ht * 128:(ht + 1) * 128, :],
                            h_sb[:, ht * TOK:(ht + 1) * TOK],
                        )
                    ag_out = pdram.tile([NCORES * H, TOK], BF16, tag="ag_out",
                                        addr_space="Shared")
                    nc.gpsimd.collective_compute(
                        "AllGather",
                        mybir.AluOpType.bypass,
                        replica_groups=RG,
                        ins=[ag_in[:].opt()],
                        outs=[ag_out[:].opt()],
                    )
                    h_full = phf.tile([128, HT * S], BF16, tag="h_full")
                    for k in range(HT):
                        for s in range(NCORES):
                            nc.sync.dma_start(
                                h_full[:, k * S + s * TOK:
                                       k * S + (s + 1) * TOK],
                                ag_out[s * H + k * 128:
                                       s * H + (k + 1) * 128, :],
                            )
                return h_full

            def reduce_scatter(o_parts):
                """o_parts: callable(m, n) -> sbuf bf16 tile [128, CH] holding
                the partial output for features m*128.., tokens n*CH..  Writes
                the s-blocked bounce, runs RS, adds the result into x_sb."""
                rs_in = pdram.tile([NCORES * H, TOK], BF16, tag="rs_in")
                for n in range(NCH):
                    for m in range(HT):
                        pt = o_parts(m, n)
                        for j in range(CH // TOK):
                            s = (n * CH) // TOK + j
                            nc.sync.dma_start(
                                rs_in[s * H + m * 128: s * H + (m + 1) * 128, :],
                                pt[:, j * TOK:(j + 1) * TOK],
                            )
                rs_out = pdram.tile([H, TOK], BF16, tag="rs_out")
                nc.gpsimd.collective_compute(
                    "ReduceScatter",
                    mybir.AluOpType.add,
                    replica_groups=RG,
                    ins=[rs_in[:].opt()],
                    outs=[rs_out[:].opt()],
                )
                for ht in range(HT):
                    radd = poutw.tile([128, TOK], BF16, tag="radd")
                    nc.sync.dma_start(radd[:],
                                      rs_out[ht * 128:(ht + 1) * 128, :])
                    nc.vector.tensor_tensor(
                        x_sb[:, ht * TOK:(ht + 1) * TOK],
                        x_sb[:, ht * TOK:(ht + 1) * TOK],
                        radd[:],
                        op=ADD,
                    )

            def rope(src, c0):
                """In-place RoPE on an S-column window of src starting at
                column c0 (feature-major [64, .] head block; cos/sin tables
                carry the sign pattern)."""
                cs = slice(c0, c0 + S)
                rot = pbig.tile([64, S], BF16, tag="ropescratch")
                nc.vector.tensor_copy(rot[0:32, :], src[32:64, cs])
                nc.vector.tensor_copy(rot[32:64, :], src[0:32, cs])
                nc.vector.tensor_tensor(
                    rot[:], rot[:], sin_sb[:, 0:S], op=MUL,
                )
                nc.vector.tensor_tensor(
                    src[:, cs], src[:, cs], cos_sb[:, 0:S], op=MUL,
                )
                nc.vector.tensor_tensor(
                    src[:, cs], src[:, cs], rot[:], op=ADD,
                )

            for l in range(L):
                w = W[l]
                # ---------------- attention ----------------
                h1 = rmsnorm_ag_load(f"a{l}")

                # weight loads (lhsT layouts)
                wq_sb = pwts.tile([128, HT * QD], BF16, tag="wq")
                wk_sb = pwts.tile([128, HT * D], BF16, tag="wk")
                wv_sb = pwts.tile([128, HT * D], BF16, tag="wv")
                wo_sb = pwts.tile([128, HT * 128], BF16, tag="wo")
                for k in range(HT):
                    nc.sync.dma_start(
                        wq_sb[:, k * QD:(k + 1) * QD],
                        w["wq"][k * 128:(k + 1) * 128, :],
                    )
                    nc.sync.dma_start(
                        wk_sb[:, k * D:(k + 1) * D],
                        w["wk"][k * 128:(k + 1) * 128, :],
                    )
                    nc.sync.dma_start(
                        wv_sb[:, k * D:(k + 1) * D],
                        w["wv"][k * 128:(k + 1) * 128, :],
                    )
                    nc.sync.dma_start(
                        wo_sb[:, k * 128:(k + 1) * 128],
                        w["wo"][:, k * 128:(k + 1) * 128],
                    )

                # q head-blocked [64, 2S]: head hh lives at cols hh*S..
                q_hb = pbig.tile([64, 2 * S], BF16, tag="q_hb")
                kTt = pbig.tile([64, S], BF16, tag="kT")
                vT = pbig.tile([64, S], BF16, tag="vT")
                with tc.tile_pool(name=f"ps_qkv{l}", bufs=2,
                                  space="PSUM") as pps:
                    for n in range(NCH):
                        pq = pps.tile([128, CH], F32, tag="pq")
                        pk = pps.tile([64, CH], F32, tag="pk")
                        pv = pps.tile([64, CH], F32, tag="pv")
                        for k in range(HT):
                            rhs = h1[:, k * S + n * CH: k * S + (n + 1) * CH]
                            nc.tensor.matmul(
                                pq[:], wq_sb[:, k * QD:(k + 1) * QD], rhs,
                                start=(k == 0), stop=(k == HT - 1),
                            )
                            nc.tensor.matmul(
                                pk[:], wk_sb[:, k * D:(k + 1) * D], rhs,
                                start=(k == 0), stop=(k == HT - 1),
                            )
                            nc.tensor.matmul(
                                pv[:], wv_sb[:, k * D:(k + 1) * D], rhs,
                                start=(k == 0), stop=(k == HT - 1),
                            )
                        ns = slice(n * CH, (n + 1) * CH)
                        nc.vector.tensor_copy(q_hb[:, ns], pq[0:64, :])
                        nc.vector.tensor_copy(
                            q_hb[:, S + n * CH:S + (n + 1) * CH],
                            pq[64:128, :],
                        )
                        nc.vector.tensor_copy(kTt[:, ns], pk[:])
                        nc.vector.tensor_copy(vT[:, ns], pv[:])

                    rope(q_hb, 0)
                    rope(q_hb, S)
                    rope(kTt, 0)

                    # token-major v with ones column (softmax sum rides attn@v)
                    v_tok = pbig.tile([128, KT * 65], BF16, tag="v_tok")
                    for kt in range(KT):
                        nc.vector.tensor_copy(
                            v_tok[:, kt * 65 + 64:kt * 65 + 65],
                            ones[:, 0:1],
                        )
                    for kt in range(KT):
                        pvt = pps.tile([128, 64], BF16, tag="pvt")
                        nc.tensor.transpose(
                            pvt[:], vT[:, kt * 128:(kt + 1) * 128],
                            ident[0:64, 0:64],
                        )
                        nc.vector.tensor_copy(
                            v_tok[:, kt * 65:kt * 65 + 64], pvt[:]
                        )

                oT = pbig.tile([128, S], BF16, tag="oT")
                with tc.tile_pool(name=f"ps_att{l}", bufs=1,
                                  space="PSUM") as ppa:
                    for hh in range(QH):
                        pav = ppa.tile([65, S], F32, tag="pav")
                        for kt in range(KT):
                            for half in range(2):
                                psc = ppa.tile([128, 2 * CH], F32, tag="psc",
                                               bufs=2)
                                for j in range(2):
                                    n = half * 2 + j
                                    nc.tensor.matmul(
                                        psc[:, j * CH:(j + 1) * CH],
                                        kTt[:, kt * 128:(kt + 1) * 128],
                                        q_hb[:, hh * S + n * CH:
                                             hh * S + (n + 1) * CH],
                                        start=True, stop=True,
                                    )
                                et = pexp.tile([128, 2 * CH], BF16, tag="et")
                                nc.scalar.activation(
                                    et[:], psc[:],
                                    mybir.ActivationFunctionType.Exp,
                                )
                                for j in range(2):
                                    n = half * 2 + j
                                    nc.tensor.matmul(
                                        pav[:, n * CH:(n + 1) * CH],
                                        v_tok[:, kt * 65:(kt + 1) * 65],
                                        et[:, j * CH:(j + 1) * CH],
                                        start=(kt == 0), stop=(kt == KT - 1),
                                        skip_group_check=True,
                                    )
                        for n in range(NCH):
                            ns = slice(n * CH, (n + 1) * CH)
                            rec = psmall.tile([1, CH], F32R, tag="rec")
                            with nc.allow_low_precision(
                                reason="f32r is fp32 bits"
                            ):
                                nc.vector.reciprocal(rec[:], pav[64:65, ns])
                            rbc = ppa.tile([64, CH], F32, tag="psc", bufs=2)
                            nc.tensor.matmul(
                                rbc[:], onesr[0:1, 0:64], rec[:],
                                start=True, stop=True,
                            )
                            rbs = poutw.tile([64, CH], F32, tag="rbs")
                            nc.vector.tensor_copy(rbs[:], rbc[:])
                            nc.vector.tensor_tensor(
                                oT[hh * 64:(hh + 1) * 64, ns],
                                pav[0:64, ns], rbs[:], op=MUL,
                            )

                # row-parallel o-proj -> RS -> residual add
                with tc.tile_pool(name=f"ps_o{l}", bufs=3,
                                  space="PSUM") as ppo:
                    def o_part(m, n, _oT=oT, _wo=wo_sb, _pp=ppo):
                        po = _pp.tile([128, CH], F32, tag="po")
                        nc.tensor.matmul(
                            po[:], _wo[:, m * 128:(m + 1) * 128],
                            _oT[:, n * CH:(n + 1) * CH],
                            start=True, stop=True,
                        )
                        ot = poutw.tile([128, CH], BF16, tag="opart")
                        nc.vector.tensor_copy(ot[:], po[:])
                        return ot

                    reduce_scatter(o_part)

                # ---------------- mlp ----------------
                h2 = rmsnorm_ag_load(f"m{l}")
                wg_sb = pwts.tile([128, HT * FFL], BF16, tag="wg")
                wu_sb = pwts.tile([128, HT * FFL], BF16, tag="wu")
                wd_sb = pwts.tile([128, 4 * H], BF16, tag="wd")
                for k in range(HT):
                    nc.sync.dma_start(
                        wg_sb[:, k * FFL:(k + 1) * FFL],
                        w["wg"][k * 128:(k + 1) * 128, :],
                    )
                    nc.sync.dma_start(
                        wu_sb[:, k * FFL:(k + 1) * FFL],
                        w["wu"][k * 128:(k + 1) * 128, :],
                    )
                for fk in range(4):
                    nc.sync.dma_start(
                        wd_sb[:, fk * H:(fk + 1) * H],
                        w["wd"][fk * 128:(fk + 1) * 128, :],
                    )

                act = pacts.tile([128, 4 * S], BF16, tag="act")
                with tc.tile_pool(name=f"ps_gu{l}", bufs=2,
                                  space="PSUM") as ppg:
                    for f in range(4):
                        for n in range(NCH):
                            pg = ppg.tile([128, CH], F32, tag="pg")
                            pu = ppg.tile([128, CH], F32, tag="pu")
                            for k in range(HT):
                                rhs = h2[:, k * S + n * CH:
                                         k * S + (n + 1) * CH]
                                nc.tensor.matmul(
                                    pg[:],
                                    wg_sb[:, k * FFL + f * 128:
                                          k * FFL + (f + 1) * 128],
                                    rhs,
                                    start=(k == 0), stop=(k == HT - 1),
                                )
                                nc.tensor.matmul(
                                    pu[:],
                                    wu_sb[:, k * FFL + f * 128:
                                          k * FFL + (f + 1) * 128],
                                    rhs,
                                    start=(k == 0), stop=(k == HT - 1),
                                )
                            ns = slice(f * S + n * CH, f * S + (n + 1) * CH)
                            nc.scalar.activation(
                                act[:, ns], pg[:],
                                mybir.ActivationFunctionType.Silu,
                            )
                            ut = poutw.tile([128, CH], BF16, tag="ut")
                            nc.vector.tensor_copy(ut[:], pu[:])
                            nc.vector.tensor_tensor(
                                act[:, ns], act[:, ns], ut[:], op=MUL,
                            )

                with tc.tile_pool(name=f"ps_d{l}", bufs=3,
                                  space="PSUM") as ppd:
                    def d_part(m, n, _act=act, _wd=wd_sb, _pp=ppd):
                        pd = _pp.tile([128, CH], F32, tag="pd")
                        for fk in range(4):
                            nc.tensor.matmul(
                                pd[:],
                                _wd[:, fk * H + m * 128:
                                    fk * H + (m + 1) * 128],
                                _act[:, fk * S + n * CH:
                                     fk * S + (n + 1) * CH],
                                start=(fk == 0), stop=(fk == 3),
                            )
                        ot = poutw.tile([128, CH], BF16, tag="opart")
                        nc.vector.tensor_copy(ot[:], pd[:])
                        return ot

                    reduce_scatter(d_part)

            # final output
            for ht in range(HT):
                nc.sync.dma_start(
                    xout[ht * 128:(ht + 1) * 128, :],
                    x_sb[:, ht * TOK:(ht + 1) * TOK],
                )
    return nc


def _get_nc():
    global _CACHED_NC
    if _CACHED_NC is None:
        _CACHED_NC = _build_nc()
    return _CACHED_NC


def _host_prep(inputs):
    """Fold ln/scale into weights, pre-transpose shards, embed gather, rope
    tables.  Returns in_maps (list of dicts, one per core)."""
    ids = np.asarray(inputs["input_ids"])[0]          # [S] int32
    embed = np.asarray(inputs["embed"], np.float32)   # [V, H]
    x = embed[ids]                                    # [S, H]

    inv = 1.0 / (10000.0 ** (np.arange(0, D, 2, dtype=np.float32) / D))  # [32]
    freqs = np.arange(S, dtype=np.float32)[:, None] * inv[None, :]       # [S,32]
    cosT = np.cos(freqs).T.astype(np.float32)   # [32, S]
    sinT = np.sin(freqs).T.astype(np.float32)
    cosF = np.ascontiguousarray(np.tile(cosT, (2, 1))).astype(BF)
    sinF = np.ascontiguousarray(np.concatenate([-sinT, sinT], 0)).astype(BF)

    def bf(a):
        return np.ascontiguousarray(a).astype(BF)

    scale = np.float32(1.0 / np.sqrt(D))
    in_maps = []
    for c in range(NCORES):
        m = {
            "x0": np.ascontiguousarray(x[c * TOK:(c + 1) * TOK, :].T),
            "cosf": cosF,
            "sinf": sinF,
        }
        for l in range(L):
            ln1 = np.asarray(inputs["ln1"], np.float32)[l]
            ln2 = np.asarray(inputs["ln2"], np.float32)[l]
            wq = np.asarray(inputs["Wq"], np.float32)[l] * ln1[None, :] * scale
            wk = np.asarray(inputs["Wk"], np.float32)[l] * ln1[None, :]
            wv = np.asarray(inputs["Wv"], np.float32)[l] * ln1[None, :]
            wo = np.asarray(inputs["Wo"], np.float32)[l]
            wg = np.asarray(inputs["Wg"], np.float32)[l] * ln2[None, :]
            wu = np.asarray(inputs["Wu"], np.float32)[l] * ln2[None, :]
            wd = np.asarray(inputs["Wd"], np.float32)[l]
            m[f"wq{l}"] = bf(wq[c * QD:(c + 1) * QD, :].T)
            m[f"wk{l}"] = bf(wk.T)
            m[f"wv{l}"] = bf(wv.T)
            m[f"wo{l}"] = bf(wo[:, c * QD:(c + 1) * QD].T)
            m[f"wg{l}"] = bf(wg[c * FFL:(c + 1) * FFL, :].T)
            m[f"wu{l}"] = bf(wu[c * FFL:(c + 1) * FFL, :].T)
            m[f"wd{l}"] = bf(wd[:, c * FFL:(c + 1) * FFL].T)
        in_maps.append(m)
    return in_maps


def kernel(**inputs) -> np.ndarray:
    nc = _get_nc()
    in_maps = _host_prep(inputs)
    res = bass_utils.run_bass_kernel_spmd(
        nc, in_maps, core_ids=list(range(NCORES))
    )
    out = np.empty((1, S, H), np.float32)
    for c in range(NCORES):
        out[0, c * TOK:(c + 1) * TOK, :] = res.results[c]["xout"].T
    return out


# revision 30
# speedup vs baseline: 1.3885x; 1.0010x over previous
"""Trainium2 Bass kernel for nn_CustomLlamaModel (2-layer MQA llama, B=1 S=2048
H=1024 HQ=16 HKV=1 FF=4096), tensor-parallel over 8 NeuronCores.

Strategy (per sharding hint): column-parallel q/gate/up, row-parallel o/down,
KV head replicated; residual stream token-sharded (256 tokens/core) and kept
feature-major [H, tok] fp32 in SBUF.  Per layer: rmsnorm (PE ones-matmul
partition reduce, fp32r) -> AllGather(h, bf16) -> qkv+rope -> transposed-
scores attention (softmax along partitions via exp + ones-column folded into
the attn@v matmul) -> row-parallel o-proj -> ReduceScatter(bf16) -> residual
add -> rmsnorm -> AllGather -> gate/up/silu -> row-parallel down ->
ReduceScatter -> residual add.  Matmul pipeline runs in bf16 (fp32 PSUM
accumulate); the gathered h stays fully SBUF-resident.  ln1/ln2 and the
1/sqrt(D) attention scale are folded into weights host-side; the embedding
gather runs host-side (pure numpy indexing).
"""
import sys

sys.path.insert(0, "/opt/trn_rl_repo")

import ml_dtypes
import numpy as np
import orjson

import concourse.bass as bass
import concourse.mybir as mybir
import concourse.tile as tile
from concourse import bass_utils
from concourse.masks import make_identity

# ---------------------------------------------------------------------------
# Walrus in this container supports only ONE sync-wait per instruction, but
# Tile's scheduler emits multi-wait instructions.  Post-process the BIR JSON:
# split each multi-wait instruction into single-wait NoOps (same engine,
# program-order before the original).
# ---------------------------------------------------------------------------
_orig_to_json_bytes = bass.Bass.to_json_bytes
_MW = [0]


def _split_multiwait(d):
    changed = False

    def fix_block(bb):
        nonlocal changed
        insts = bb.get("instructions")
        if not insts:
            return
        out = []
        for ins in insts:
            si = ins.get("sync_info")
            if si:
                ow = si.get("on_wait") or []
                if len(ow) > 1:
                    changed = True
                    for w in ow[:-1]:
                        _MW[0] += 1
                        out.append({
                            "debug": ins.get("debug", 0),
                            "engine": ins["engine"],
                            "ins": [],
                            "outs": [],
                            "name": f"{ins['name']}-mw{_MW[0]}",
                            "opcode": "NoOp",
                            "sync_info": {"on_update": [], "on_wait": [w]},
                        })
                    si["on_wait"] = [ow[-1]]
            out.append(ins)
        bb["instructions"] = out

    def rec(o):
        if isinstance(o, dict):
            if isinstance(o.get("instructions"), list):
                fix_block(o)
            for v in o.values():
                rec(v)
        elif isinstance(o, list):
            for v in o:
                rec(v)

    for fn in d.get("functions", []):
        rec(fn)
    return changed


def _patched_to_json_bytes(self):
    raw = _orig_to_json_bytes(self)
    d = orjson.loads(raw)
    if _split_multiwait(d):
        return orjson.dumps(d)
    return raw


bass.Bass.to_json_bytes = _patched_to_json_bytes

# ---------------------------------------------------------------------------
# Model / sharding constants
# ---------------------------------------------------------------------------
S, H, D, HQ, FF, L, V = 2048, 1024, 64, 16, 4096, 2, 32000
EPS = 1e-6
NCORES = 8
TOK = S // NCORES       # residual tokens per core (256)
QH = HQ // NCORES       # heads per core (2)
QD = QH * D             # q dims per core (128)
FFL = FF // NCORES      # ff dims per core (512)
NCH = 4                 # token chunks of 512
CH = S // NCH           # 512
KT = S // 128           # 16 key-token tiles
HT = H // 128           # 8 hidden feature tiles
F32 = mybir.dt.float32
F32R = mybir.dt.float32r
BF16 = mybir.dt.bfloat16
MUL = mybir.AluOpType.mult
ADD = mybir.AluOpType.add
RG = [list(range(NCORES))]
BF = ml_dtypes.bfloat16

_CACHED_NC = None


def _build_nc():
    nc = bass.Bass()
    x0 = nc.dram_tensor("x0", [H, TOK], F32, kind="ExternalInput")
    cosf = nc.dram_tensor("cosf", [64, S], BF16, kind="ExternalInput")
    sinf = nc.dram_tensor("sinf", [64, S], BF16, kind="ExternalInput")
    W = []
    for l in range(L):
        W.append({
            "wq": nc.dram_tensor(f"wq{l}", [H, QD], BF16, kind="ExternalInput"),
            "wk": nc.dram_tensor(f"wk{l}", [H, D], BF16, kind="ExternalInput"),
            "wv": nc.dram_tensor(f"wv{l}", [H, D], BF16, kind="ExternalInput"),
            "wo": nc.dram_tensor(f"wo{l}", [QD, H], BF16, kind="ExternalInput"),
            "wg": nc.dram_tensor(f"wg{l}", [H, FFL], BF16, kind="ExternalInput"),
            "wu": nc.dram_tensor(f"wu{l}", [H, FFL], BF16, kind="ExternalInput"),
            "wd": nc.dram_tensor(f"wd{l}", [FFL, H], BF16, kind="ExternalInput"),
        })
    xout = nc.dram_tensor("xout", [H, TOK], F32, kind="ExternalOutput")

    with tile.TileContext(nc) as tc:
        with (
            tc.tile_pool(name="const", bufs=1) as pconst,
            tc.tile_pool(name="resid", bufs=1) as presid,
            tc.tile_pool(name="wts", bufs=1) as pwts,
            tc.tile_pool(name="hfull", bufs=1) as phf,
            tc.tile_pool(name="acts", bufs=1) as pacts,
            tc.tile_pool(name="big", bufs=1) as pbig,
            tc.tile_pool(name="small", bufs=2) as psmall,
            tc.tile_pool(name="exp", bufs=2) as pexp,
            tc.tile_pool(name="outw", bufs=3) as poutw,
            tc.tile_pool(name="dram", bufs=2, space="DRAM") as pdram,
        ):
            # constants
            identf = pconst.tile([128, 128], F32, tag="identf")
            make_identity(nc, identf[:])
            ident = pconst.tile([128, 128], BF16, tag="ident")
            nc.vector.tensor_copy(ident[:], identf[:])
            onesf = pconst.tile([128, 128], F32, tag="onesf")
            nc.vector.memset(onesf[:], 1.0)
            ones = pconst.tile([128, 128], BF16, tag="ones")
            nc.vector.tensor_copy(ones[:], onesf[:])
            onesr = pconst.tile([128, 128], F32R, tag="onesr")
            nc.vector.tensor_copy(onesr[:], onesf[:])
            epst = pconst.tile([128, 1], F32, tag="eps")
            nc.gpsimd.memset(epst[:], EPS)
            cos_sb = pconst.tile([64, S], BF16, tag="cos")
            sin_sb = pconst.tile([64, S], BF16, tag="sin")
            nc.sync.dma_start(cos_sb[:], cosf[:])
            nc.sync.dma_start(sin_sb[:], sinf[:])

            # residual x, feature-major: block ht -> cols [ht*TOK:(ht+1)*TOK]
            x_sb = presid.tile([128, HT * TOK], F32, tag="x")
            for ht in range(HT):
                nc.sync.dma_start(
                    x_sb[:, ht * TOK:(ht + 1) * TOK],
                    x0[ht * 128:(ht + 1) * 128, :],
                )

            def rmsnorm_ag_load(tag):
                """x_sb -> normalized h (bf16) -> bounce -> AllGather (two
                feature-half chunks, pipelined) -> load into a fully-resident
                SBUF tile h_full [128, HT*S] (block k = features k*128..,
                cols = tokens in order)."""
                with tc.tile_pool(name=f"ps_n_{tag}", bufs=2,
                                  space="PSUM") as pps:
                    x2 = pbig.tile([128, HT * TOK], F32R, tag="rot")
                    nc.vector.tensor_tensor(x2[:], x_sb[:], x_sb[:], op=MUL)
                    ssq = pps.tile([1, TOK], F32, tag="ssq")
                    for ht in range(HT):
                        nc.tensor.matmul(
                            ssq[:],
                            onesr[:, 0:1],
                            x2[:, ht * TOK:(ht + 1) * TOK],
                            start=(ht == 0),
                            stop=(ht == HT - 1),
                        )
                    sstd = psmall.tile([1, TOK], F32, tag="sstd")
                    nc.scalar.activation(
                        sstd[:], ssq[:], mybir.ActivationFunctionType.Sqrt,
                        bias=epst[0:1, :], scale=1.0 / H,
                    )
                    rinv = psmall.tile([1, TOK], F32R, tag="rinv")
                    with nc.allow_low_precision(reason="f32r is fp32 bits"):
                        nc.vector.reciprocal(rinv[:], sstd[:])
                    rb = pps.tile([128, TOK], F32, tag="rb")
                    nc.tensor.matmul(rb[:], onesr[0:1, :], rinv[:],
                                     start=True, stop=True)
                    h_sb = pbig.tile([128, HT * TOK], BF16, tag="h")
                    h_full = phf.tile([128, HT * S], BF16, tag="h_full")
                    HH = HT // 2  # feature tiles per chunk
                    for j in range(2):
                        for ht in range(j * HH, (j + 1) * HH):
                            nc.vector.tensor_tensor(
                                h_sb[:, ht * TOK:(ht + 1) * TOK],
                                x_sb[:, ht * TOK:(ht + 1) * TOK],
                                rb[:],
                                op=MUL,
                            )
                        ag_in = pdram.tile([HH * 128, TOK], BF16,
                                           tag=f"ag_in{j}")
                        for ht in range(HH):
                            nc.sync.dma_start(
                                ag_in[ht * 128:(ht + 1) * 128, :],
                                h_sb[:, (j * HH + ht) * TOK:
                                     (j * HH + ht + 1) * TOK],
                            )
                        ag_out = pdram.tile([NCORES * HH * 128, TOK], BF16,
                                            tag=f"ag_out{j}",
                                            addr_space="Shared")
                        nc.gpsimd.collective_compute(
                            "AllGather",
                            mybir.AluOpType.bypass,
                            replica_groups=RG,
                            ins=[ag_in[:].opt()],
                            outs=[ag_out[:].opt()],
                        )
                        for k in range(HH):
                            for s in range(NCORES):
                                nc.sync.dma_start(
                                    h_full[:, (j * HH + k) * S + s * TOK:
                                           (j * HH + k) * S + (s + 1) * TOK],
                                    ag_out[s * HH * 128 + k * 128:
                                           s * HH * 128 + (k + 1) * 128, :],
                                )
                return h_full

            def reduce_scatter(o_parts):
                """o_parts: callable(m, n) -> sbuf bf16 tile [128, CH] holding
                the partial output for features m*128.., tokens n*CH..
                Feature-chunked (2 halves): RS of chunk 0 flies while the
                m=4..7 projections still run.  Adds results into x_sb."""
                HH = HT // 2
                for j in range(2):
                    rs_in = pdram.tile([NCORES * HH * 128, TOK], BF16,
                                       tag=f"rs_in{j}")
                    for m in range(j * HH, (j + 1) * HH):
                        for n in range(NCH):
                            pt = o_parts(m, n)
                            for jj in range(CH // TOK):
                                s = (n * CH) // TOK + jj
                                nc.sync.dma_start(
                                    rs_in[s * HH * 128 + (m - j * HH) * 128:
                                          s * HH * 128 + (m - j * HH + 1) * 128,
                                          :],
                                    pt[:, jj * TOK:(jj + 1) * TOK],
                                )
                    rs_out = pdram.tile([HH * 128, TOK], BF16, tag=f"rs_out{j}")
                    nc.gpsimd.collective_compute(
                        "ReduceScatter",
                        mybir.AluOpType.add,
                        replica_groups=RG,
                        ins=[rs_in[:].opt()],
                        outs=[rs_out[:].opt()],
                    )
                    for ht in range(HH):
                        radd = poutw.tile([128, TOK], BF16, tag="radd")
                        nc.sync.dma_start(
                            radd[:], rs_out[ht * 128:(ht + 1) * 128, :]
                        )
                        nc.vector.tensor_tensor(
                            x_sb[:, (j * HH + ht) * TOK:
                                 (j * HH + ht + 1) * TOK],
                            x_sb[:, (j * HH + ht) * TOK:
                                 (j * HH + ht + 1) * TOK],
                            radd[:],
                            op=ADD,
                        )

            def rope(src, c0):
                """In-place RoPE on an S-column window of src starting at
                column c0 (feature-major [64, .] head block; cos/sin tables
                carry the sign pattern)."""
                cs = slice(c0, c0 + S)
                rot = pbig.tile([64, S], BF16, tag="ropescratch")
                nc.vector.tensor_copy(rot[0:32, :], src[32:64, cs])
                nc.vector.tensor_copy(rot[32:64, :], src[0:32, cs])
                nc.vector.tensor_tensor(
                    rot[:], rot[:], sin_sb[:, 0:S], op=MUL,
                )
                nc.vector.tensor_tensor(
                    src[:, cs], src[:, cs], cos_sb[:, 0:S], op=MUL,
                )
                nc.vector.tensor_tensor(
                    src[:, cs], src[:, cs], rot[:], op=ADD,
                )

            for l in range(L):
                w = W[l]
                # ---------------- attention ----------------
                h1 = rmsnorm_ag_load(f"a{l}")

                # weight loads (lhsT layouts)
                wq_sb = pwts.tile([128, HT * QD], BF16, tag="wq")
                wk_sb = pwts.tile([128, HT * D], BF16, tag="wk")
                wv_sb = pwts.tile([128, HT * D], BF16, tag="wv")
                wo_sb = pwts.tile([128, HT * 128], BF16, tag="wo")
                for k in range(HT):
                    nc.sync.dma_start(
                        wq_sb[:, k * QD:(k + 1) * QD],
                        w["wq"][k * 128:(k + 1) * 128, :],
                    )
                    nc.sync.dma_start(
                        wk_sb[:, k * D:(k + 1) * D],
                        w["wk"][k * 128:(k + 1) * 128, :],
                    )
                    nc.sync.dma_start(
                        wv_sb[:, k * D:(k + 1) * D],
                        w["wv"][k * 128:(k + 1) * 128, :],
                    )
                    nc.sync.dma_start(
                        wo_sb[:, k * 128:(k + 1) * 128],
                        w["wo"][:, k * 128:(k + 1) * 128],
                    )

                # q head-blocked [64, 2S]: head hh lives at cols hh*S..
                q_hb = pbig.tile([64, 2 * S], BF16, tag="q_hb")
                kTt = pbig.tile([64, S], BF16, tag="kT")
                vT = pbig.tile([64, S], BF16, tag="vT")
                with tc.tile_pool(name=f"ps_qkv{l}", bufs=2,
                                  space="PSUM") as pps:
                    for n in range(NCH):
                        pq = pps.tile([128, CH], F32, tag="pq")
                        pk = pps.tile([64, CH], F32, tag="pk")
                        pv = pps.tile([64, CH], F32, tag="pv")
                        for k in range(HT):
                            rhs = h1[:, k * S + n * CH: k * S + (n + 1) * CH]
                            nc.tensor.matmul(
                                pq[:], wq_sb[:, k * QD:(k + 1) * QD], rhs,
                                start=(k == 0), stop=(k == HT - 1),
                            )
                            nc.tensor.matmul(
                                pk[:], wk_sb[:, k * D:(k + 1) * D], rhs,
                                start=(k == 0), stop=(k == HT - 1),
                            )
                            nc.tensor.matmul(
                                pv[:], wv_sb[:, k * D:(k + 1) * D], rhs,
                                start=(k == 0), stop=(k == HT - 1),
                            )
                        ns = slice(n * CH, (n + 1) * CH)
                        nc.vector.tensor_copy(q_hb[:, ns], pq[0:64, :])
                        nc.vector.tensor_copy(
                            q_hb[:, S + n * CH:S + (n + 1) * CH],
                            pq[64:128, :],
                        )
                        nc.vector.tensor_copy(kTt[:, ns], pk[:])
                        nc.vector.tensor_copy(vT[:, ns], pv[:])

                    rope(q_hb, 0)
                    rope(q_hb, S)
                    rope(kTt, 0)

                    # token-major v with ones column (softmax sum rides attn@v)
                    v_tok = pbig.tile([128, KT * 65], BF16, tag="v_tok")
                    for kt in range(KT):
                        nc.vector.tensor_copy(
                            v_tok[:, kt * 65 + 64:kt * 65 + 65],
                            ones[:, 0:1],
                        )
                    for kt in range(KT):
                        pvt = pps.tile([128, 64], BF16, tag="pvt")
                        nc.tensor.transpose(
                            pvt[:], vT[:, kt * 128:(kt + 1) * 128],
                            ident[0:64, 0:64],
                        )
                        nc.vector.tensor_copy(
                            v_tok[:, kt * 65:kt * 65 + 64], pvt[:]
                        )

                oT = pbig.tile([128, S], BF16, tag="oT")
                with tc.tile_pool(name=f"ps_att{l}", bufs=1,
                                  space="PSUM") as ppa:
                    for hh in range(QH):
                        for qh in range(2):
                            q0 = qh * (S // 2)        # q-column base
                            pav = ppa.tile([65, S // 2], F32, tag="pav",
                                           bufs=1)
                            for kt in range(KT):
                                psc = ppa.tile([128, 2 * CH], F32, tag="psc",
                                               bufs=3)
                                for j in range(2):
                                    nc.tensor.matmul(
                                        psc[:, j * CH:(j + 1) * CH],
                                        kTt[:, kt * 128:(kt + 1) * 128],
                                        q_hb[:, hh * S + q0 + j * CH:
                                             hh * S + q0 + (j + 1) * CH],
                                        start=True, stop=True,
                                    )
                                et = pexp.tile([128, 2 * CH], BF16, tag="et")
                                nc.scalar.activation(
                                    et[:], psc[:],
                                    mybir.ActivationFunctionType.Exp,
                                )
                                for j in range(2):
                                    nc.tensor.matmul(
                                        pav[:, j * CH:(j + 1) * CH],
                                        v_tok[:, kt * 65:(kt + 1) * 65],
                                        et[:, j * CH:(j + 1) * CH],
                                        start=(kt == 0), stop=(kt == KT - 1),
                                        skip_group_check=True,
                                    )
                            for j in range(2):
                                js = slice(j * CH, (j + 1) * CH)
                                rec = psmall.tile([1, CH], F32R, tag="rec")
                                with nc.allow_low_precision(
                                    reason="f32r is fp32 bits"
                                ):
                                    nc.vector.reciprocal(rec[:],
                                                         pav[64:65, js])
                                rbc = ppa.tile([64, CH], F32, tag="psc",
                                               bufs=3)
                                nc.tensor.matmul(
                                    rbc[:], onesr[0:1, 0:64], rec[:],
                                    start=True, stop=True,
                                )
                                rbs = poutw.tile([64, CH], F32, tag="rbs")
                                nc.vector.tensor_copy(rbs[:], rbc[:])
                                nc.vector.tensor_tensor(
                                    oT[hh * 64:(hh + 1) * 64,
                                       q0 + j * CH:q0 + (j + 1) * CH],
                                    pav[0:64, js], rbs[:], op=MUL,
                                )

                # row-parallel o-proj -> RS -> residual add
                with tc.tile_pool(name=f"ps_o{l}", bufs=3,
                                  space="PSUM") as ppo:
                    def o_part(m, n, _oT=oT, _wo=wo_sb, _pp=ppo):
                        po = _pp.tile([128, CH], F32, tag="po")
                        nc.tensor.matmul(
                            po[:], _wo[:, m * 128:(m + 1) * 128],
                            _oT[:, n * CH:(n + 1) * CH],
                            start=True, stop=True,
                        )
                        ot = poutw.tile([128, CH], BF16, tag="opart")
                        nc.vector.tensor_copy(ot[:], po[:])
                        return ot

                    reduce_scatter(o_part)

                # ---------------- mlp ----------------
                h2 = rmsnorm_ag_load(f"m{l}")
                wg_sb = pwts.tile([128, HT * FFL], BF16, tag="wg")
                wu_sb = pwts.tile([128, HT * FFL], BF16, tag="wu")
                wd_sb = pwts.tile([128, 4 * H], BF16, tag="wd")
                for k in range(HT):
                    nc.sync.dma_start(
                        wg_sb[:, k * FFL:(k + 1) * FFL],
                        w["wg"][k * 128:(k + 1) * 128, :],
                    )
                    nc.sync.dma_start(
                        wu_sb[:, k * FFL:(k + 1) * FFL],
                        w["wu"][k * 128:(k + 1) * 128, :],
                    )
                for fk in range(4):
                    nc.sync.dma_start(
                        wd_sb[:, fk * H:(fk + 1) * H],
                        w["wd"][fk * 128:(fk + 1) * 128, :],
                    )

                act = pacts.tile([128, 4 * S], BF16, tag="act")
                with tc.tile_pool(name=f"ps_gu{l}", bufs=2,
                                  space="PSUM") as ppg:
                    for f in range(4):
                        for n in range(NCH):
                            pg = ppg.tile([128, CH], F32, tag="pg")
                            pu = ppg.tile([128, CH], F32, tag="pu")
                            for k in range(HT):
                                rhs = h2[:, k * S + n * CH:
                                         k * S + (n + 1) * CH]
                                nc.tensor.matmul(
                                    pg[:],
                                    wg_sb[:, k * FFL + f * 128:
                                          k * FFL + (f + 1) * 128],
                                    rhs,
                                    start=(k == 0), stop=(k == HT - 1),
                                )
                                nc.tensor.matmul(
                                    pu[:],
                                    wu_sb[:, k * FFL + f * 128:
                                          k * FFL + (f + 1) * 128],
                                    rhs,
                                    start=(k == 0), stop=(k == HT - 1),
                                )
                            ns = slice(f * S + n * CH, f * S + (n + 1) * CH)
                            nc.scalar.activation(
                                act[:, ns], pg[:],
                                mybir.ActivationFunctionType.Silu,
                            )
                            ut = poutw.tile([128, CH], BF16, tag="ut")
                            nc.vector.tensor_copy(ut[:], pu[:])
                            nc.vector.tensor_tensor(
                                act[:, ns], act[:, ns], ut[:], op=MUL,
                            )

                with tc.tile_pool(name=f"ps_d{l}", bufs=3,
                                  space="PSUM") as ppd:
                    def d_part(m, n, _act=act, _wd=wd_sb, _pp=ppd):
                        pd = _pp.tile([128, CH], F32, tag="pd")
                        for fk in range(4):
                            nc.tensor.matmul(
                                pd[:],
                                _wd[:, fk * H + m * 128:
                                    fk * H + (m + 1) * 128],
                                _act[:, fk * S + n * CH:
                                     fk * S + (n + 1) * CH],
                                start=(fk == 0), stop=(fk == 3),
                            )
                        ot = poutw.tile([128, CH], BF16, tag="opart")
                        nc.vector.tensor_copy(ot[:], pd[:])
                        return ot

                    reduce_scatter(d_part)

            # final output
            for ht in range(HT):
                nc.sync.dma_start(
                    xout[ht * 128:(ht + 1) * 128, :],
                    x_sb[:, ht * TOK:(ht + 1) * TOK],
                )
    return nc


def _get_nc():
    global _CACHED_NC
    if _CACHED_NC is None:
        _CACHED_NC = _build_nc()
    return _CACHED_NC


def _host_prep(inputs):
    """Fold ln/scale into weights, pre-transpose shards, embed gather, rope
    tables.  Returns in_maps (list of dicts, one per core)."""
    ids = np.asarray(inputs["input_ids"])[0]          # [S] int32
    embed = np.asarray(inputs["embed"], np.float32)   # [V, H]
    x = embed[ids]                                    # [S, H]

    inv = 1.0 / (10000.0 ** (np.arange(0, D, 2, dtype=np.float32) / D))  # [32]
    freqs = np.arange(S, dtype=np.float32)[:, None] * inv[None, :]       # [S,32]
    cosT = np.cos(freqs).T.astype(np.float32)   # [32, S]
    sinT = np.sin(freqs).T.astype(np.float32)
    cosF = np.ascontiguousarray(np.tile(cosT, (2, 1))).astype(BF)
    sinF = np.ascontiguousarray(np.concatenate([-sinT, sinT], 0)).astype(BF)

    def bf(a):
        return np.ascontiguousarray(a).astype(BF)

    scale = np.float32(1.0 / np.sqrt(D))
    in_maps = []
    for c in range(NCORES):
        m = {
            "x0": np.ascontiguousarray(x[c * TOK:(c + 1) * TOK, :].T),
            "cosf": cosF,
            "sinf": sinF,
        }
        for l in range(L):
            ln1 = np.asarray(inputs["ln1"], np.float32)[l]
            ln2 = np.asarray(inputs["ln2"], np.float32)[l]
            wq = np.asarray(inputs["Wq"], np.float32)[l] * ln1[None, :] * scale
            wk = np.asarray(inputs["Wk"], np.float32)[l] * ln1[None, :]
            wv = np.asarray(inputs["Wv"], np.float32)[l] * ln1[None, :]
            wo = np.asarray(inputs["Wo"], np.float32)[l]
            wg = np.asarray(inputs["Wg"], np.float32)[l] * ln2[None, :]
            wu = np.asarray(inputs["Wu"], np.float32)[l] * ln2[None, :]
            wd = np.asarray(inputs["Wd"], np.float32)[l]
            m[f"wq{l}"] = bf(wq[c * QD:(c + 1) * QD, :].T)
            m[f"wk{l}"] = bf(wk.T)
            m[f"wv{l}"] = bf(wv.T)
            m[f"wo{l}"] = bf(wo[:, c * QD:(c + 1) * QD].T)
            m[f"wg{l}"] = bf(wg[c * FFL:(c + 1) * FFL, :].T)
            m[f"wu{l}"] = bf(wu[c * FFL:(c + 1) * FFL, :].T)
            m[f"wd{l}"] = bf(wd[:, c * FFL:(c + 1) * FFL].T)
        in_maps.append(m)
    return in_maps


def kernel(**inputs) -> np.ndarray:
    nc = _get_nc()
    in_maps = _host_prep(inputs)
    res = bass_utils.run_bass_kernel_spmd(
        nc, in_maps, core_ids=list(range(NCORES))
    )
    out = np.empty((1, S, H), np.float32)
    for c in range(NCORES):
        out[0, c * TOK:(c + 1) * TOK, :] = res.results[c]["xout"].T
    return out


# revision 31
# speedup vs baseline: 1.3905x; 1.0015x over previous
"""Trainium2 Bass kernel for nn_CustomLlamaModel (2-layer MQA llama, B=1 S=2048
H=1024 HQ=16 HKV=1 FF=4096), tensor-parallel over 8 NeuronCores.

Strategy (per sharding hint): column-parallel q/gate/up, row-parallel o/down,
KV head replicated; residual stream token-sharded (256 tokens/core) and kept
feature-major [H, tok] fp32 in SBUF.  Per layer: rmsnorm (PE ones-matmul
partition reduce, fp32r) -> AllGather(h, bf16) -> qkv+rope -> transposed-
scores attention (softmax along partitions via exp + ones-column folded into
the attn@v matmul) -> row-parallel o-proj -> ReduceScatter(bf16) -> residual
add -> rmsnorm -> AllGather -> gate/up/silu -> row-parallel down ->
ReduceScatter -> residual add.  Matmul pipeline runs in bf16 (fp32 PSUM
accumulate); the gathered h stays fully SBUF-resident.  ln1/ln2 and the
1/sqrt(D) attention scale are folded into weights host-side; the embedding
gather runs host-side (pure numpy indexing).
"""
import sys

sys.path.insert(0, "/opt/trn_rl_repo")

import ml_dtypes
import numpy as np
import orjson

import concourse.bass as bass
import concourse.mybir as mybir
import concourse.tile as tile
from concourse import bass_utils
from concourse.masks import make_identity

# ---------------------------------------------------------------------------
# Walrus in this container supports only ONE sync-wait per instruction, but
# Tile's scheduler emits multi-wait instructions.  Post-process the BIR JSON:
# split each multi-wait instruction into single-wait NoOps (same engine,
# program-order before the original).
# ---------------------------------------------------------------------------
_orig_to_json_bytes = bass.Bass.to_json_bytes
_MW = [0]


def _split_multiwait(d):
    changed = False

    def fix_block(bb):
        nonlocal changed
        insts = bb.get("instructions")
        if not insts:
            return
        out = []
        for ins in insts:
            si = ins.get("sync_info")
            if si:
                ow = si.get("on_wait") or []
                if len(ow) > 1:
                    changed = True
                    for w in ow[:-1]:
                        _MW[0] += 1
                        out.append({
                            "debug": ins.get("debug", 0),
                            "engine": ins["engine"],
                            "ins": [],
                            "outs": [],
                            "name": f"{ins['name']}-mw{_MW[0]}",
                            "opcode": "NoOp",
                            "sync_info": {"on_update": [], "on_wait": [w]},
                        })
                    si["on_wait"] = [ow[-1]]
            out.append(ins)
        bb["instructions"] = out

    def rec(o):
        if isinstance(o, dict):
            if isinstance(o.get("instructions"), list):
                fix_block(o)
            for v in o.values():
                rec(v)
        elif isinstance(o, list):
            for v in o:
                rec(v)

    for fn in d.get("functions", []):
        rec(fn)
    return changed


def _patched_to_json_bytes(self):
    raw = _orig_to_json_bytes(self)
    d = orjson.loads(raw)
    if _split_multiwait(d):
        return orjson.dumps(d)
    return raw


bass.Bass.to_json_bytes = _patched_to_json_bytes

# ---------------------------------------------------------------------------
# Model / sharding constants
# ---------------------------------------------------------------------------
S, H, D, HQ, FF, L, V = 2048, 1024, 64, 16, 4096, 2, 32000
EPS = 1e-6
NCORES = 8
TOK = S // NCORES       # residual tokens per core (256)
QH = HQ // NCORES       # heads per core (2)
QD = QH * D             # q dims per core (128)
FFL = FF // NCORES      # ff dims per core (512)
NCH = 4                 # token chunks of 512
CH = S // NCH           # 512
KT = S // 128           # 16 key-token tiles
HT = H // 128           # 8 hidden feature tiles
F32 = mybir.dt.float32
F32R = mybir.dt.float32r
BF16 = mybir.dt.bfloat16
MUL = mybir.AluOpType.mult
ADD = mybir.AluOpType.add
RG = [list(range(NCORES))]
BF = ml_dtypes.bfloat16

_CACHED_NC = None


def _build_nc():
    nc = bass.Bass()
    x0 = nc.dram_tensor("x0", [H, TOK], F32, kind="ExternalInput")
    cosf = nc.dram_tensor("cosf", [64, S], BF16, kind="ExternalInput")
    sinf = nc.dram_tensor("sinf", [64, S], BF16, kind="ExternalInput")
    W = []
    for l in range(L):
        W.append({
            "wq": nc.dram_tensor(f"wq{l}", [H, QD], BF16, kind="ExternalInput"),
            "wk": nc.dram_tensor(f"wk{l}", [H, D], BF16, kind="ExternalInput"),
            "wv": nc.dram_tensor(f"wv{l}", [H, D], BF16, kind="ExternalInput"),
            "wo": nc.dram_tensor(f"wo{l}", [QD, H], BF16, kind="ExternalInput"),
            "wg": nc.dram_tensor(f"wg{l}", [H, FFL], BF16, kind="ExternalInput"),
            "wu": nc.dram_tensor(f"wu{l}", [H, FFL], BF16, kind="ExternalInput"),
            "wd": nc.dram_tensor(f"wd{l}", [FFL, H], BF16, kind="ExternalInput"),
        })
    xout = nc.dram_tensor("xout", [H, TOK], F32, kind="ExternalOutput")

    with tile.TileContext(nc) as tc:
        with (
            tc.tile_pool(name="const", bufs=1) as pconst,
            tc.tile_pool(name="resid", bufs=1) as presid,
            tc.tile_pool(name="wts", bufs=1) as pwts,
            tc.tile_pool(name="hfull", bufs=1) as phf,
            tc.tile_pool(name="acts", bufs=1) as pacts,
            tc.tile_pool(name="big", bufs=1) as pbig,
            tc.tile_pool(name="small", bufs=2) as psmall,
            tc.tile_pool(name="exp", bufs=2) as pexp,
            tc.tile_pool(name="outw", bufs=3) as poutw,
            tc.tile_pool(name="dram", bufs=2, space="DRAM") as pdram,
        ):
            # constants
            identf = pconst.tile([128, 128], F32, tag="identf")
            make_identity(nc, identf[:])
            ident = pconst.tile([128, 128], BF16, tag="ident")
            nc.vector.tensor_copy(ident[:], identf[:])
            onesf = pconst.tile([128, 128], F32, tag="onesf")
            nc.vector.memset(onesf[:], 1.0)
            ones = pconst.tile([128, 128], BF16, tag="ones")
            nc.vector.tensor_copy(ones[:], onesf[:])
            onesr = pconst.tile([128, 128], F32R, tag="onesr")
            nc.vector.tensor_copy(onesr[:], onesf[:])
            epst = pconst.tile([128, 1], F32, tag="eps")
            nc.gpsimd.memset(epst[:], EPS)
            cos_sb = pconst.tile([64, S], BF16, tag="cos")
            sin_sb = pconst.tile([64, S], BF16, tag="sin")
            nc.sync.dma_start(cos_sb[:], cosf[:])
            nc.sync.dma_start(sin_sb[:], sinf[:])

            # residual x, feature-major: block ht -> cols [ht*TOK:(ht+1)*TOK]
            x_sb = presid.tile([128, HT * TOK], F32, tag="x")
            for ht in range(HT):
                nc.sync.dma_start(
                    x_sb[:, ht * TOK:(ht + 1) * TOK],
                    x0[ht * 128:(ht + 1) * 128, :],
                )

            def rmsnorm_ag_load(tag):
                """x_sb -> normalized h (bf16) -> bounce -> AllGather (two
                feature-half chunks, pipelined) -> load into a fully-resident
                SBUF tile h_full [128, HT*S] (block k = features k*128..,
                cols = tokens in order)."""
                with tc.tile_pool(name=f"ps_n_{tag}", bufs=2,
                                  space="PSUM") as pps:
                    x2 = pbig.tile([128, HT * TOK], F32R, tag="rot")
                    nc.vector.tensor_tensor(x2[:], x_sb[:], x_sb[:], op=MUL)
                    ssq = pps.tile([1, TOK], F32, tag="ssq")
                    for ht in range(HT):
                        nc.tensor.matmul(
                            ssq[:],
                            onesr[:, 0:1],
                            x2[:, ht * TOK:(ht + 1) * TOK],
                            start=(ht == 0),
                            stop=(ht == HT - 1),
                        )
                    sstd = psmall.tile([1, TOK], F32, tag="sstd")
                    nc.scalar.activation(
                        sstd[:], ssq[:], mybir.ActivationFunctionType.Sqrt,
                        bias=epst[0:1, :], scale=1.0 / H,
                    )
                    rinv = psmall.tile([1, TOK], F32R, tag="rinv")
                    with nc.allow_low_precision(reason="f32r is fp32 bits"):
                        nc.vector.reciprocal(rinv[:], sstd[:])
                    rb = pps.tile([128, TOK], F32, tag="rb")
                    nc.tensor.matmul(rb[:], onesr[0:1, :], rinv[:],
                                     start=True, stop=True)
                    h_sb = pbig.tile([128, HT * TOK], BF16, tag="h")
                    h_full = phf.tile([128, HT * S], BF16, tag="h_full")
                    HH = HT  # single chunk (collective floors dominate)
                    for j in range(1):
                        for ht in range(j * HH, (j + 1) * HH):
                            nc.vector.tensor_tensor(
                                h_sb[:, ht * TOK:(ht + 1) * TOK],
                                x_sb[:, ht * TOK:(ht + 1) * TOK],
                                rb[:],
                                op=MUL,
                            )
                        ag_in = pdram.tile([HH * 128, TOK], BF16,
                                           tag=f"ag_in{j}")
                        for ht in range(HH):
                            nc.sync.dma_start(
                                ag_in[ht * 128:(ht + 1) * 128, :],
                                h_sb[:, (j * HH + ht) * TOK:
                                     (j * HH + ht + 1) * TOK],
                            )
                        ag_out = pdram.tile([NCORES * HH * 128, TOK], BF16,
                                            tag=f"ag_out{j}",
                                            addr_space="Shared")
                        nc.gpsimd.collective_compute(
                            "AllGather",
                            mybir.AluOpType.bypass,
                            replica_groups=RG,
                            ins=[ag_in[:].opt()],
                            outs=[ag_out[:].opt()],
                        )
                        for k in range(HH):
                            for s in range(NCORES):
                                nc.sync.dma_start(
                                    h_full[:, (j * HH + k) * S + s * TOK:
                                           (j * HH + k) * S + (s + 1) * TOK],
                                    ag_out[s * HH * 128 + k * 128:
                                           s * HH * 128 + (k + 1) * 128, :],
                                )
                return h_full

            def reduce_scatter(o_parts):
                """o_parts: callable(m, n) -> sbuf bf16 tile [128, CH] holding
                the partial output for features m*128.., tokens n*CH..
                Feature-chunked (2 halves): RS of chunk 0 flies while the
                m=4..7 projections still run.  Adds results into x_sb."""
                HH = HT
                for j in range(1):
                    rs_in = pdram.tile([NCORES * HH * 128, TOK], BF16,
                                       tag=f"rs_in{j}")
                    for m in range(j * HH, (j + 1) * HH):
                        for n in range(NCH):
                            pt = o_parts(m, n)
                            for jj in range(CH // TOK):
                                s = (n * CH) // TOK + jj
                                nc.sync.dma_start(
                                    rs_in[s * HH * 128 + (m - j * HH) * 128:
                                          s * HH * 128 + (m - j * HH + 1) * 128,
                                          :],
                                    pt[:, jj * TOK:(jj + 1) * TOK],
                                )
                    rs_out = pdram.tile([HH * 128, TOK], BF16, tag=f"rs_out{j}")
                    nc.gpsimd.collective_compute(
                        "ReduceScatter",
                        mybir.AluOpType.add,
                        replica_groups=RG,
                        ins=[rs_in[:].opt()],
                        outs=[rs_out[:].opt()],
                    )
                    for ht in range(HH):
                        radd = poutw.tile([128, TOK], BF16, tag="radd")
                        nc.sync.dma_start(
                            radd[:], rs_out[ht * 128:(ht + 1) * 128, :]
                        )
                        nc.vector.tensor_tensor(
                            x_sb[:, (j * HH + ht) * TOK:
                                 (j * HH + ht + 1) * TOK],
                            x_sb[:, (j * HH + ht) * TOK:
                                 (j * HH + ht + 1) * TOK],
                            radd[:],
                            op=ADD,
                        )

            def rope(src, c0):
                """In-place RoPE on an S-column window of src starting at
                column c0 (feature-major [64, .] head block; cos/sin tables
                carry the sign pattern)."""
                cs = slice(c0, c0 + S)
                rot = pbig.tile([64, S], BF16, tag="ropescratch")
                nc.vector.tensor_copy(rot[0:32, :], src[32:64, cs])
                nc.vector.tensor_copy(rot[32:64, :], src[0:32, cs])
                nc.vector.tensor_tensor(
                    rot[:], rot[:], sin_sb[:, 0:S], op=MUL,
                )
                nc.vector.tensor_tensor(
                    src[:, cs], src[:, cs], cos_sb[:, 0:S], op=MUL,
                )
                nc.vector.tensor_tensor(
                    src[:, cs], src[:, cs], rot[:], op=ADD,
                )

            for l in range(L):
                w = W[l]
                # ---------------- attention ----------------
                h1 = rmsnorm_ag_load(f"a{l}")

                # weight loads (lhsT layouts)
                wq_sb = pwts.tile([128, HT * QD], BF16, tag="wq")
                wk_sb = pwts.tile([128, HT * D], BF16, tag="wk")
                wv_sb = pwts.tile([128, HT * D], BF16, tag="wv")
                wo_sb = pwts.tile([128, HT * 128], BF16, tag="wo")
                for k in range(HT):
                    nc.sync.dma_start(
                        wq_sb[:, k * QD:(k + 1) * QD],
                        w["wq"][k * 128:(k + 1) * 128, :],
                    )
                    nc.sync.dma_start(
                        wk_sb[:, k * D:(k + 1) * D],
                        w["wk"][k * 128:(k + 1) * 128, :],
                    )
                    nc.sync.dma_start(
                        wv_sb[:, k * D:(k + 1) * D],
                        w["wv"][k * 128:(k + 1) * 128, :],
                    )
                    nc.sync.dma_start(
                        wo_sb[:, k * 128:(k + 1) * 128],
                        w["wo"][:, k * 128:(k + 1) * 128],
                    )

                # q head-blocked [64, 2S]: head hh lives at cols hh*S..
                q_hb = pbig.tile([64, 2 * S], BF16, tag="q_hb")
                kTt = pbig.tile([64, S], BF16, tag="kT")
                vT = pbig.tile([64, S], BF16, tag="vT")
                with tc.tile_pool(name=f"ps_qkv{l}", bufs=2,
                                  space="PSUM") as pps:
                    for n in range(NCH):
                        pq = pps.tile([128, CH], F32, tag="pq")
                        pk = pps.tile([64, CH], F32, tag="pk")
                        pv = pps.tile([64, CH], F32, tag="pv")
                        for k in range(HT):
                            rhs = h1[:, k * S + n * CH: k * S + (n + 1) * CH]
                            nc.tensor.matmul(
                                pq[:], wq_sb[:, k * QD:(k + 1) * QD], rhs,
                                start=(k == 0), stop=(k == HT - 1),
                            )
                            nc.tensor.matmul(
                                pk[:], wk_sb[:, k * D:(k + 1) * D], rhs,
                                start=(k == 0), stop=(k == HT - 1),
                            )
                            nc.tensor.matmul(
                                pv[:], wv_sb[:, k * D:(k + 1) * D], rhs,
                                start=(k == 0), stop=(k == HT - 1),
                            )
                        ns = slice(n * CH, (n + 1) * CH)
                        nc.vector.tensor_copy(q_hb[:, ns], pq[0:64, :])
                        nc.vector.tensor_copy(
                            q_hb[:, S + n * CH:S + (n + 1) * CH],
                            pq[64:128, :],
                        )
                        nc.vector.tensor_copy(kTt[:, ns], pk[:])
                        nc.vector.tensor_copy(vT[:, ns], pv[:])

                    rope(q_hb, 0)
                    rope(q_hb, S)
                    rope(kTt, 0)

                    # token-major v with ones column (softmax sum rides attn@v)
                    v_tok = pbig.tile([128, KT * 65], BF16, tag="v_tok")
                    for kt in range(KT):
                        nc.vector.tensor_copy(
                            v_tok[:, kt * 65 + 64:kt * 65 + 65],
                            ones[:, 0:1],
                        )
                    for kt in range(KT):
                        pvt = pps.tile([128, 64], BF16, tag="pvt")
                        nc.tensor.transpose(
                            pvt[:], vT[:, kt * 128:(kt + 1) * 128],
                            ident[0:64, 0:64],
                        )
                        nc.vector.tensor_copy(
                            v_tok[:, kt * 65:kt * 65 + 64], pvt[:]
                        )

                oT = pbig.tile([128, S], BF16, tag="oT")
                with tc.tile_pool(name=f"ps_att{l}", bufs=1,
                                  space="PSUM") as ppa:
                    for hh in range(QH):
                        for qh in range(2):
                            q0 = qh * (S // 2)        # q-column base
                            pav = ppa.tile([65, S // 2], F32, tag="pav",
                                           bufs=1)
                            for kt in range(KT):
                                psc = ppa.tile([128, 2 * CH], F32, tag="psc",
                                               bufs=3)
                                for j in range(2):
                                    nc.tensor.matmul(
                                        psc[:, j * CH:(j + 1) * CH],
                                        kTt[:, kt * 128:(kt + 1) * 128],
                                        q_hb[:, hh * S + q0 + j * CH:
                                             hh * S + q0 + (j + 1) * CH],
                                        start=True, stop=True,
                                    )
                                et = pexp.tile([128, 2 * CH], BF16, tag="et")
                                nc.scalar.activation(
                                    et[:], psc[:],
                                    mybir.ActivationFunctionType.Exp,
                                )
                                for j in range(2):
                                    nc.tensor.matmul(
                                        pav[:, j * CH:(j + 1) * CH],
                                        v_tok[:, kt * 65:(kt + 1) * 65],
                                        et[:, j * CH:(j + 1) * CH],
                                        start=(kt == 0), stop=(kt == KT - 1),
                                        skip_group_check=True,
                                    )
                            for j in range(2):
                                js = slice(j * CH, (j + 1) * CH)
                                rec = psmall.tile([1, CH], F32R, tag="rec")
                                with nc.allow_low_precision(
                                    reason="f32r is fp32 bits"
                                ):
                                    nc.vector.reciprocal(rec[:],
                                                         pav[64:65, js])
                                rbc = ppa.tile([64, CH], F32, tag="psc",
                                               bufs=3)
                                nc.tensor.matmul(
                                    rbc[:], onesr[0:1, 0:64], rec[:],
                                    start=True, stop=True,
                                )
                                rbs = poutw.tile([64, CH], F32, tag="rbs")
                                nc.vector.tensor_copy(rbs[:], rbc[:])
                                nc.vector.tensor_tensor(
                                    oT[hh * 64:(hh + 1) * 64,
                                       q0 + j * CH:q0 + (j + 1) * CH],
                                    pav[0:64, js], rbs[:], op=MUL,
                                )

                # row-parallel o-proj -> RS -> residual add
                with tc.tile_pool(name=f"ps_o{l}", bufs=3,
                                  space="PSUM") as ppo:
                    def o_part(m, n, _oT=oT, _wo=wo_sb, _pp=ppo):
                        po = _pp.tile([128, CH], F32, tag="po")
                        nc.tensor.matmul(
                            po[:], _wo[:, m * 128:(m + 1) * 128],
                            _oT[:, n * CH:(n + 1) * CH],
                            start=True, stop=True,
                        )
                        ot = poutw.tile([128, CH], BF16, tag="opart")
                        nc.vector.tensor_copy(ot[:], po[:])
                        return ot

                    reduce_scatter(o_part)

                # ---------------- mlp ----------------
                h2 = rmsnorm_ag_load(f"m{l}")
                wg_sb = pwts.tile([128, HT * FFL], BF16, tag="wg")
                wu_sb = pwts.tile([128, HT * FFL], BF16, tag="wu")
                wd_sb = pwts.tile([128, 4 * H], BF16, tag="wd")
                for k in range(HT):
                    nc.sync.dma_start(
                        wg_sb[:, k * FFL:(k + 1) * FFL],
                        w["wg"][k * 128:(k + 1) * 128, :],
                    )
                    nc.sync.dma_start(
                        wu_sb[:, k * FFL:(k + 1) * FFL],
                        w["wu"][k * 128:(k + 1) * 128, :],
                    )
                for fk in range(4):
                    nc.sync.dma_start(
                        wd_sb[:, fk * H:(fk + 1) * H],
                        w["wd"][fk * 128:(fk + 1) * 128, :],
                    )

                act = pacts.tile([128, 4 * S], BF16, tag="act")
                with tc.tile_pool(name=f"ps_gu{l}", bufs=2,
                                  space="PSUM") as ppg:
                    for f in range(4):
                        for n in range(NCH):
                            pg = ppg.tile([128, CH], F32, tag="pg")
                            pu = ppg.tile([128, CH], F32, tag="pu")
                            for k in range(HT):
                                rhs = h2[:, k * S + n * CH:
                                         k * S + (n + 1) * CH]
                                nc.tensor.matmul(
                                    pg[:],
                                    wg_sb[:, k * FFL + f * 128:
                                          k * FFL + (f + 1) * 128],
                                    rhs,
                                    start=(k == 0), stop=(k == HT - 1),
                                )
                                nc.tensor.matmul(
                                    pu[:],
                                    wu_sb[:, k * FFL + f * 128:
                                          k * FFL + (f + 1) * 128],
                                    rhs,
                                    start=(k == 0), stop=(k == HT - 1),
                                )
                            ns = slice(f * S + n * CH, f * S + (n + 1) * CH)
                            nc.scalar.activation(
                                act[:, ns], pg[:],
                                mybir.ActivationFunctionType.Silu,
                            )
                            ut = poutw.tile([128, CH], BF16, tag="ut")
                            nc.vector.tensor_copy(ut[:], pu[:])
                            nc.vector.tensor_tensor(
                                act[:, ns], act[:, ns], ut[:], op=MUL,
                            )

                with tc.tile_pool(name=f"ps_d{l}", bufs=3,
                                  space="PSUM") as ppd:
                    def d_part(m, n, _act=act, _wd=wd_sb, _pp=ppd):
                        pd = _pp.tile([128, CH], F32, tag="pd")
                        for fk in range(4):
                            nc.tensor.matmul(
                                pd[:],
                                _wd[:, fk * H + m * 128:
                                    fk * H + (m + 1) * 128],
                                _act[:, fk * S + n * CH:
                                     fk * S + (n + 1) * CH],
                                start=(fk == 0), stop=(fk == 3),
                            )
                        ot = poutw.tile([128, CH], BF16, tag="opart")
                        nc.vector.tensor_copy(ot[:], pd[:])
                        return ot

                    reduce_scatter(d_part)

            # final output
            for ht in range(HT):
                nc.sync.dma_start(
                    xout[ht * 128:(ht + 1) * 128, :],
                    x_sb[:, ht * TOK:(ht + 1) * TOK],
                )
    return nc


def _get_nc():
    global _CACHED_NC
    if _CACHED_NC is None:
        _CACHED_NC = _build_nc()
    return _CACHED_NC


def _host_prep(inputs):
    """Fold ln/scale into weights, pre-transpose shards, embed gather, rope
    tables.  Returns in_maps (list of dicts, one per core)."""
    ids = np.asarray(inputs["input_ids"])[0]          # [S] int32
    embed = np.asarray(inputs["embed"], np.float32)   # [V, H]
    x = embed[ids]                                    # [S, H]

    inv = 1.0 / (10000.0 ** (np.arange(0, D, 2, dtype=np.float32) / D))  # [32]
    freqs = np.arange(S, dtype=np.float32)[:, None] * inv[None, :]       # [S,32]
    cosT = np.cos(freqs).T.astype(np.float32)   # [32, S]
    sinT = np.sin(freqs).T.astype(np.float32)
    cosF = np.ascontiguousarray(np.tile(cosT, (2, 1))).astype(BF)
    sinF = np.ascontiguousarray(np.concatenate([-sinT, sinT], 0)).astype(BF)

    def bf(a):
        return np.ascontiguousarray(a).astype(BF)

    scale = np.float32(1.0 / np.sqrt(D))
    in_maps = []
    for c in range(NCORES):
        m = {
            "x0": np.ascontiguousarray(x[c * TOK:(c + 1) * TOK, :].T),
            "cosf": cosF,
            "sinf": sinF,
        }
        for l in range(L):
            ln1 = np.asarray(inputs["ln1"], np.float32)[l]
            ln2 = np.asarray(inputs["ln2"], np.float32)[l]
            wq = np.asarray(inputs["Wq"], np.float32)[l] * ln1[None, :] * scale
            wk = np.asarray(inputs["Wk"], np.float32)[l] * ln1[None, :]
            wv = np.asarray(inputs["Wv"], np.float32)[l] * ln1[None, :]
            wo = np.asarray(inputs["Wo"], np.float32)[l]
            wg = np.asarray(inputs["Wg"], np.float32)[l] * ln2[None, :]
            wu = np.asarray(inputs["Wu"], np.float32)[l] * ln2[None, :]
            wd = np.asarray(inputs["Wd"], np.float32)[l]
            m[f"wq{l}"] = bf(wq[c * QD:(c + 1) * QD, :].T)
            m[f"wk{l}"] = bf(wk.T)
            m[f"wv{l}"] = bf(wv.T)
            m[f"wo{l}"] = bf(wo[:, c * QD:(c + 1) * QD].T)
            m[f"wg{l}"] = bf(wg[c * FFL:(c + 1) * FFL, :].T)
            m[f"wu{l}"] = bf(wu[c * FFL:(c + 1) * FFL, :].T)
            m[f"wd{l}"] = bf(wd[:, c * FFL:(c + 1) * FFL].T)
        in_maps.append(m)
    return in_maps


def kernel(**inputs) -> np.ndarray:
    nc = _get_nc()
    in_maps = _host_prep(inputs)
    res = bass_utils.run_bass_kernel_spmd(
        nc, in_maps, core_ids=list(range(NCORES))
    )
    out = np.empty((1, S, H), np.float32)
    for c in range(NCORES):
        out[0, c * TOK:(c + 1) * TOK, :] = res.results[c]["xout"].T
    return out


# revision 32
# speedup vs baseline: 1.3907x; 1.0002x over previous
"""Trainium2 Bass kernel for nn_CustomLlamaModel (2-layer MQA llama, B=1 S=2048
H=1024 HQ=16 HKV=1 FF=4096), tensor-parallel over 8 NeuronCores.

Strategy (per sharding hint): column-parallel q/gate/up, row-parallel o/down,
KV head replicated; residual stream token-sharded (256 tokens/core) and kept
feature-major [H, tok] fp32 in SBUF.  Per layer: rmsnorm (PE ones-matmul
partition reduce, fp32r) -> AllGather(h, bf16) -> qkv+rope -> transposed-
scores attention (softmax along partitions via exp + ones-column folded into
the attn@v matmul) -> row-parallel o-proj -> ReduceScatter(bf16) -> residual
add -> rmsnorm -> AllGather -> gate/up/silu -> row-parallel down ->
ReduceScatter -> residual add.  Matmul pipeline runs in bf16 (fp32 PSUM
accumulate); the gathered h stays fully SBUF-resident.  ln1/ln2 and the
1/sqrt(D) attention scale are folded into weights host-side; the embedding
gather runs host-side (pure numpy indexing).
"""
import sys

sys.path.insert(0, "/opt/trn_rl_repo")

import ml_dtypes
import numpy as np
import orjson

import concourse.bass as bass
import concourse.mybir as mybir
import concourse.tile as tile
from concourse import bass_utils
from concourse.masks import make_identity

# ---------------------------------------------------------------------------
# Walrus in this container supports only ONE sync-wait per instruction, but
# Tile's scheduler emits multi-wait instructions.  Post-process the BIR JSON:
# split each multi-wait instruction into single-wait NoOps (same engine,
# program-order before the original).
# ---------------------------------------------------------------------------
_orig_to_json_bytes = bass.Bass.to_json_bytes
_MW = [0]


def _split_multiwait(d):
    changed = False

    def fix_block(bb):
        nonlocal changed
        insts = bb.get("instructions")
        if not insts:
            return
        out = []
        for ins in insts:
            si = ins.get("sync_info")
            if si:
                ow = si.get("on_wait") or []
                if len(ow) > 1:
                    changed = True
                    for w in ow[:-1]:
                        _MW[0] += 1
                        out.append({
                            "debug": ins.get("debug", 0),
                            "engine": ins["engine"],
                            "ins": [],
                            "outs": [],
                            "name": f"{ins['name']}-mw{_MW[0]}",
                            "opcode": "NoOp",
                            "sync_info": {"on_update": [], "on_wait": [w]},
                        })
                    si["on_wait"] = [ow[-1]]
            out.append(ins)
        bb["instructions"] = out

    def rec(o):
        if isinstance(o, dict):
            if isinstance(o.get("instructions"), list):
                fix_block(o)
            for v in o.values():
                rec(v)
        elif isinstance(o, list):
            for v in o:
                rec(v)

    for fn in d.get("functions", []):
        rec(fn)
    return changed


def _patched_to_json_bytes(self):
    raw = _orig_to_json_bytes(self)
    d = orjson.loads(raw)
    if _split_multiwait(d):
        return orjson.dumps(d)
    return raw


bass.Bass.to_json_bytes = _patched_to_json_bytes

# ---------------------------------------------------------------------------
# Model / sharding constants
# ---------------------------------------------------------------------------
S, H, D, HQ, FF, L, V = 2048, 1024, 64, 16, 4096, 2, 32000
EPS = 1e-6
NCORES = 8
TOK = S // NCORES       # residual tokens per core (256)
QH = HQ // NCORES       # heads per core (2)
QD = QH * D             # q dims per core (128)
FFL = FF // NCORES      # ff dims per core (512)
NCH = 4                 # token chunks of 512
CH = S // NCH           # 512
KT = S // 128           # 16 key-token tiles
HT = H // 128           # 8 hidden feature tiles
F32 = mybir.dt.float32
F32R = mybir.dt.float32r
BF16 = mybir.dt.bfloat16
MUL = mybir.AluOpType.mult
ADD = mybir.AluOpType.add
RG = [list(range(NCORES))]
BF = ml_dtypes.bfloat16

_CACHED_NC = None


def _build_nc():
    nc = bass.Bass()
    x0 = nc.dram_tensor("x0", [H, TOK], F32, kind="ExternalInput")
    cosf = nc.dram_tensor("cosf", [64, S], BF16, kind="ExternalInput")
    sinf = nc.dram_tensor("sinf", [64, S], BF16, kind="ExternalInput")
    W = []
    for l in range(L):
        W.append({
            "wq": nc.dram_tensor(f"wq{l}", [H, QD], BF16, kind="ExternalInput"),
            "wk": nc.dram_tensor(f"wk{l}", [H, D], BF16, kind="ExternalInput"),
            "wv": nc.dram_tensor(f"wv{l}", [H, D], BF16, kind="ExternalInput"),
            "wo": nc.dram_tensor(f"wo{l}", [QD, H], BF16, kind="ExternalInput"),
            "wg": nc.dram_tensor(f"wg{l}", [H, FFL], BF16, kind="ExternalInput"),
            "wu": nc.dram_tensor(f"wu{l}", [H, FFL], BF16, kind="ExternalInput"),
            "wd": nc.dram_tensor(f"wd{l}", [FFL, H], BF16, kind="ExternalInput"),
        })
    xout = nc.dram_tensor("xout", [H, TOK], F32, kind="ExternalOutput")

    with tile.TileContext(nc) as tc:
        with (
            tc.tile_pool(name="const", bufs=1) as pconst,
            tc.tile_pool(name="resid", bufs=1) as presid,
            tc.tile_pool(name="wts", bufs=1) as pwts,
            tc.tile_pool(name="hfull", bufs=1) as phf,
            tc.tile_pool(name="acts", bufs=1) as pacts,
            tc.tile_pool(name="big", bufs=1) as pbig,
            tc.tile_pool(name="small", bufs=2) as psmall,
            tc.tile_pool(name="exp", bufs=3) as pexp,
            tc.tile_pool(name="outw", bufs=3) as poutw,
            tc.tile_pool(name="dram", bufs=2, space="DRAM") as pdram,
        ):
            # constants
            identf = pconst.tile([128, 128], F32, tag="identf")
            make_identity(nc, identf[:])
            ident = pconst.tile([128, 128], BF16, tag="ident")
            nc.vector.tensor_copy(ident[:], identf[:])
            onesf = pconst.tile([128, 128], F32, tag="onesf")
            nc.vector.memset(onesf[:], 1.0)
            ones = pconst.tile([128, 128], BF16, tag="ones")
            nc.vector.tensor_copy(ones[:], onesf[:])
            onesr = pconst.tile([128, 128], F32R, tag="onesr")
            nc.vector.tensor_copy(onesr[:], onesf[:])
            epst = pconst.tile([128, 1], F32, tag="eps")
            nc.gpsimd.memset(epst[:], EPS)
            cos_sb = pconst.tile([64, S], BF16, tag="cos")
            sin_sb = pconst.tile([64, S], BF16, tag="sin")
            nc.sync.dma_start(cos_sb[:], cosf[:])
            nc.sync.dma_start(sin_sb[:], sinf[:])

            # warmup: absorb the first-collective setup cost while the
            # x0 load and first norm run
            wrm_i = pdram.tile([128, 16], BF16, tag="warm_i")
            wrm_o = pdram.tile([NCORES * 128, 16], BF16, tag="warm_o",
                               addr_space="Shared")
            nc.gpsimd.collective_compute(
                "AllGather", mybir.AluOpType.bypass, replica_groups=RG,
                ins=[wrm_i[:].opt()], outs=[wrm_o[:].opt()],
            )

            # residual x, feature-major: block ht -> cols [ht*TOK:(ht+1)*TOK]
            x_sb = presid.tile([128, HT * TOK], F32, tag="x")
            for ht in range(HT):
                nc.sync.dma_start(
                    x_sb[:, ht * TOK:(ht + 1) * TOK],
                    x0[ht * 128:(ht + 1) * 128, :],
                )

            def rmsnorm_ag_load(tag):
                """x_sb -> normalized h (bf16) -> bounce -> AllGather (two
                feature-half chunks, pipelined) -> load into a fully-resident
                SBUF tile h_full [128, HT*S] (block k = features k*128..,
                cols = tokens in order)."""
                with tc.tile_pool(name=f"ps_n_{tag}", bufs=2,
                                  space="PSUM") as pps:
                    x2 = pbig.tile([128, HT * TOK], F32R, tag="rot")
                    ssq = pps.tile([1, TOK], F32, tag="ssq")
                    for ht in range(HT):
                        hs = slice(ht * TOK, (ht + 1) * TOK)
                        nc.vector.tensor_tensor(x2[:, hs], x_sb[:, hs],
                                                x_sb[:, hs], op=MUL)
                        nc.tensor.matmul(
                            ssq[:],
                            onesr[:, 0:1],
                            x2[:, ht * TOK:(ht + 1) * TOK],
                            start=(ht == 0),
                            stop=(ht == HT - 1),
                        )
                    sstd = psmall.tile([1, TOK], F32, tag="sstd")
                    nc.scalar.activation(
                        sstd[:], ssq[:], mybir.ActivationFunctionType.Sqrt,
                        bias=epst[0:1, :], scale=1.0 / H,
                    )
                    rinv = psmall.tile([1, TOK], F32R, tag="rinv")
                    with nc.allow_low_precision(reason="f32r is fp32 bits"):
                        nc.vector.reciprocal(rinv[:], sstd[:])
                    rb = pps.tile([128, TOK], F32, tag="rb")
                    nc.tensor.matmul(rb[:], onesr[0:1, :], rinv[:],
                                     start=True, stop=True)
                    h_sb = pbig.tile([128, HT * TOK], BF16, tag="h")
                    h_full = phf.tile([128, HT * S], BF16, tag="h_full")
                    HH = HT  # single chunk (collective floors dominate)
                    for j in range(1):
                        for ht in range(j * HH, (j + 1) * HH):
                            nc.vector.tensor_tensor(
                                h_sb[:, ht * TOK:(ht + 1) * TOK],
                                x_sb[:, ht * TOK:(ht + 1) * TOK],
                                rb[:],
                                op=MUL,
                            )
                        ag_in = pdram.tile([HH * 128, TOK], BF16,
                                           tag=f"ag_in{j}")
                        for ht in range(HH):
                            nc.sync.dma_start(
                                ag_in[ht * 128:(ht + 1) * 128, :],
                                h_sb[:, (j * HH + ht) * TOK:
                                     (j * HH + ht + 1) * TOK],
                            )
                        ag_out = pdram.tile([NCORES * HH * 128, TOK], BF16,
                                            tag=f"ag_out{j}",
                                            addr_space="Shared")
                        nc.gpsimd.collective_compute(
                            "AllGather",
                            mybir.AluOpType.bypass,
                            replica_groups=RG,
                            ins=[ag_in[:].opt()],
                            outs=[ag_out[:].opt()],
                        )
                        for k in range(HH):
                            for s in range(NCORES):
                                nc.sync.dma_start(
                                    h_full[:, (j * HH + k) * S + s * TOK:
                                           (j * HH + k) * S + (s + 1) * TOK],
                                    ag_out[s * HH * 128 + k * 128:
                                           s * HH * 128 + (k + 1) * 128, :],
                                )
                return h_full

            def reduce_scatter(o_parts):
                """o_parts: callable(m, n) -> sbuf bf16 tile [128, CH] holding
                the partial output for features m*128.., tokens n*CH..
                Feature-chunked (2 halves): RS of chunk 0 flies while the
                m=4..7 projections still run.  Adds results into x_sb."""
                HH = HT
                for j in range(1):
                    rs_in = pdram.tile([NCORES * HH * 128, TOK], BF16,
                                       tag=f"rs_in{j}")
                    for m in range(j * HH, (j + 1) * HH):
                        for n in range(NCH):
                            pt = o_parts(m, n)
                            for jj in range(CH // TOK):
                                s = (n * CH) // TOK + jj
                                nc.sync.dma_start(
                                    rs_in[s * HH * 128 + (m - j * HH) * 128:
                                          s * HH * 128 + (m - j * HH + 1) * 128,
                                          :],
                                    pt[:, jj * TOK:(jj + 1) * TOK],
                                )
                    rs_out = pdram.tile([HH * 128, TOK], BF16, tag=f"rs_out{j}")
                    nc.gpsimd.collective_compute(
                        "ReduceScatter",
                        mybir.AluOpType.add,
                        replica_groups=RG,
                        ins=[rs_in[:].opt()],
                        outs=[rs_out[:].opt()],
                    )
                    for ht in range(HH):
                        radd = poutw.tile([128, TOK], BF16, tag="radd")
                        nc.sync.dma_start(
                            radd[:], rs_out[ht * 128:(ht + 1) * 128, :]
                        )
                        nc.vector.tensor_tensor(
                            x_sb[:, (j * HH + ht) * TOK:
                                 (j * HH + ht + 1) * TOK],
                            x_sb[:, (j * HH + ht) * TOK:
                                 (j * HH + ht + 1) * TOK],
                            radd[:],
                            op=ADD,
                        )

            def rope(src, c0):
                """In-place RoPE on an S-column window of src starting at
                column c0 (feature-major [64, .] head block; cos/sin tables
                carry the sign pattern)."""
                cs = slice(c0, c0 + S)
                rot = pbig.tile([64, S], BF16, tag="ropescratch")
                nc.vector.tensor_copy(rot[0:32, :], src[32:64, cs])
                nc.vector.tensor_copy(rot[32:64, :], src[0:32, cs])
                nc.vector.tensor_tensor(
                    rot[:], rot[:], sin_sb[:, 0:S], op=MUL,
                )
                nc.vector.tensor_tensor(
                    src[:, cs], src[:, cs], cos_sb[:, 0:S], op=MUL,
                )
                nc.vector.tensor_tensor(
                    src[:, cs], src[:, cs], rot[:], op=ADD,
                )

            for l in range(L):
                w = W[l]
                # ---------------- attention ----------------
                h1 = rmsnorm_ag_load(f"a{l}")

                # weight loads (lhsT layouts)
                wq_sb = pwts.tile([128, HT * QD], BF16, tag="wq")
                wk_sb = pwts.tile([128, HT * D], BF16, tag="wk")
                wv_sb = pwts.tile([128, HT * D], BF16, tag="wv")
                wo_sb = pwts.tile([128, HT * 128], BF16, tag="wo")
                for k in range(HT):
                    nc.sync.dma_start(
                        wq_sb[:, k * QD:(k + 1) * QD],
                        w["wq"][k * 128:(k + 1) * 128, :],
                    )
                    nc.sync.dma_start(
                        wk_sb[:, k * D:(k + 1) * D],
                        w["wk"][k * 128:(k + 1) * 128, :],
                    )
                    nc.sync.dma_start(
                        wv_sb[:, k * D:(k + 1) * D],
                        w["wv"][k * 128:(k + 1) * 128, :],
                    )
                    nc.sync.dma_start(
                        wo_sb[:, k * 128:(k + 1) * 128],
                        w["wo"][:, k * 128:(k + 1) * 128],
                    )

                # q head-blocked [64, 2S]: head hh lives at cols hh*S..
                q_hb = pbig.tile([64, 2 * S], BF16, tag="q_hb")
                kTt = pbig.tile([64, S], BF16, tag="kT")
                vT = pbig.tile([64, S], BF16, tag="vT")
                with tc.tile_pool(name=f"ps_qkv{l}", bufs=2,
                                  space="PSUM") as pps:
                    for n in range(NCH):
                        pq = pps.tile([128, CH], F32, tag="pq")
                        pk = pps.tile([64, CH], F32, tag="pk")
                        pv = pps.tile([64, CH], F32, tag="pv")
                        for k in range(HT):
                            rhs = h1[:, k * S + n * CH: k * S + (n + 1) * CH]
                            nc.tensor.matmul(
                                pq[:], wq_sb[:, k * QD:(k + 1) * QD], rhs,
                                start=(k == 0), stop=(k == HT - 1),
                            )
                            nc.tensor.matmul(
                                pk[:], wk_sb[:, k * D:(k + 1) * D], rhs,
                                start=(k == 0), stop=(k == HT - 1),
                            )
                            nc.tensor.matmul(
                                pv[:], wv_sb[:, k * D:(k + 1) * D], rhs,
                                start=(k == 0), stop=(k == HT - 1),
                            )
                        ns = slice(n * CH, (n + 1) * CH)
                        nc.vector.tensor_copy(q_hb[:, ns], pq[0:64, :])
                        nc.vector.tensor_copy(
                            q_hb[:, S + n * CH:S + (n + 1) * CH],
                            pq[64:128, :],
                        )
                        nc.vector.tensor_copy(kTt[:, ns], pk[:])
                        nc.vector.tensor_copy(vT[:, ns], pv[:])

                    rope(q_hb, 0)
                    rope(q_hb, S)
                    rope(kTt, 0)

                    # token-major v with ones column (softmax sum rides attn@v)
                    v_tok = pbig.tile([128, KT * 65], BF16, tag="v_tok")
                    for kt in range(KT):
                        nc.vector.tensor_copy(
                            v_tok[:, kt * 65 + 64:kt * 65 + 65],
                            ones[:, 0:1],
                        )
                    for kt in range(KT):
                        pvt = pps.tile([128, 64], BF16, tag="pvt")
                        nc.tensor.transpose(
                            pvt[:], vT[:, kt * 128:(kt + 1) * 128],
                            ident[0:64, 0:64],
                        )
                        nc.vector.tensor_copy(
                            v_tok[:, kt * 65:kt * 65 + 64], pvt[:]
                        )

                oT = pbig.tile([128, S], BF16, tag="oT")
                with tc.tile_pool(name=f"ps_att{l}", bufs=1,
                                  space="PSUM") as ppa:
                    for hh in range(QH):
                        for qh in range(2):
                            q0 = qh * (S // 2)        # q-column base
                            pav = ppa.tile([65, S // 2], F32, tag="pav",
                                           bufs=1)
                            for kt in range(KT):
                                psc = ppa.tile([128, 2 * CH], F32, tag="psc",
                                               bufs=3)
                                for j in range(2):
                                    nc.tensor.matmul(
                                        psc[:, j * CH:(j + 1) * CH],
                                        kTt[:, kt * 128:(kt + 1) * 128],
                                        q_hb[:, hh * S + q0 + j * CH:
                                             hh * S + q0 + (j + 1) * CH],
                                        start=True, stop=True,
                                    )
                                et = pexp.tile([128, 2 * CH], BF16, tag="et")
                                nc.scalar.activation(
                                    et[:], psc[:],
                                    mybir.ActivationFunctionType.Exp,
                                )
                                for j in range(2):
                                    nc.tensor.matmul(
                                        pav[:, j * CH:(j + 1) * CH],
                                        v_tok[:, kt * 65:(kt + 1) * 65],
                                        et[:, j * CH:(j + 1) * CH],
                                        start=(kt == 0), stop=(kt == KT - 1),
                                        skip_group_check=True,
                                    )
                            for j in range(2):
                                js = slice(j * CH, (j + 1) * CH)
                                rec = psmall.tile([1, CH], F32R, tag="rec")
                                with nc.allow_low_precision(
                                    reason="f32r is fp32 bits"
                                ):
                                    nc.vector.reciprocal(rec[:],
                                                         pav[64:65, js])
                                rbc = ppa.tile([64, CH], F32, tag="psc",
                                               bufs=3)
                                nc.tensor.matmul(
                                    rbc[:], onesr[0:1, 0:64], rec[:],
                                    start=True, stop=True,
                                )
                                rbs = poutw.tile([64, CH], F32, tag="rbs")
                                nc.vector.tensor_copy(rbs[:], rbc[:])
                                nc.vector.tensor_tensor(
                                    oT[hh * 64:(hh + 1) * 64,
                                       q0 + j * CH:q0 + (j + 1) * CH],
                                    pav[0:64, js], rbs[:], op=MUL,
                                )

                # row-parallel o-proj -> RS -> residual add
                with tc.tile_pool(name=f"ps_o{l}", bufs=3,
                                  space="PSUM") as ppo:
                    def o_part(m, n, _oT=oT, _wo=wo_sb, _pp=ppo):
                        po = _pp.tile([128, CH], F32, tag="po")
                        nc.tensor.matmul(
                            po[:], _wo[:, m * 128:(m + 1) * 128],
                            _oT[:, n * CH:(n + 1) * CH],
                            start=True, stop=True,
                        )
                        ot = poutw.tile([128, CH], BF16, tag="opart")
                        nc.vector.tensor_copy(ot[:], po[:])
                        return ot

                    reduce_scatter(o_part)

                # ---------------- mlp ----------------
                h2 = rmsnorm_ag_load(f"m{l}")
                wg_sb = pwts.tile([128, HT * FFL], BF16, tag="wg")
                wu_sb = pwts.tile([128, HT * FFL], BF16, tag="wu")
                wd_sb = pwts.tile([128, 4 * H], BF16, tag="wd")
                for k in range(HT):
                    nc.sync.dma_start(
                        wg_sb[:, k * FFL:(k + 1) * FFL],
                        w["wg"][k * 128:(k + 1) * 128, :],
                    )
                    nc.sync.dma_start(
                        wu_sb[:, k * FFL:(k + 1) * FFL],
                        w["wu"][k * 128:(k + 1) * 128, :],
                    )
                for fk in range(4):
                    nc.sync.dma_start(
                        wd_sb[:, fk * H:(fk + 1) * H],
                        w["wd"][fk * 128:(fk + 1) * 128, :],
                    )

                act = pacts.tile([128, 4 * S], BF16, tag="act")
                with tc.tile_pool(name=f"ps_gu{l}", bufs=2,
                                  space="PSUM") as ppg:
                    for f in range(4):
                        for n in range(NCH):
                            pg = ppg.tile([128, CH], F32, tag="pg")
                            pu = ppg.tile([128, CH], F32, tag="pu")
                            for k in range(HT):
                                rhs = h2[:, k * S + n * CH:
                                         k * S + (n + 1) * CH]
                                nc.tensor.matmul(
                                    pg[:],
                                    wg_sb[:, k * FFL + f * 128:
                                          k * FFL + (f + 1) * 128],
                                    rhs,
                                    start=(k == 0), stop=(k == HT - 1),
                                )
                                nc.tensor.matmul(
                                    pu[:],
                                    wu_sb[:, k * FFL + f * 128:
                                          k * FFL + (f + 1) * 128],
                                    rhs,
                                    start=(k == 0), stop=(k == HT - 1),
                                )
                            ns = slice(f * S + n * CH, f * S + (n + 1) * CH)
                            nc.scalar.activation(
                                act[:, ns], pg[:],
                                mybir.ActivationFunctionType.Silu,
                            )
                            ut = poutw.tile([128, CH], BF16, tag="ut")
                            nc.vector.tensor_copy(ut[:], pu[:])
                            nc.vector.tensor_tensor(
                                act[:, ns], act[:, ns], ut[:], op=MUL,
                            )

                with tc.tile_pool(name=f"ps_d{l}", bufs=3,
                                  space="PSUM") as ppd:
                    def d_part(m, n, _act=act, _wd=wd_sb, _pp=ppd):
                        pd = _pp.tile([128, CH], F32, tag="pd")
                        for fk in range(4):
                            nc.tensor.matmul(
                                pd[:],
                                _wd[:, fk * H + m * 128:
                                    fk * H + (m + 1) * 128],
                                _act[:, fk * S + n * CH:
                                     fk * S + (n + 1) * CH],
                                start=(fk == 0), stop=(fk == 3),
                            )
                        ot = poutw.tile([128, CH], BF16, tag="opart")
                        nc.vector.tensor_copy(ot[:], pd[:])
                        return ot

                    reduce_scatter(d_part)

            # final output
            for ht in range(HT):
                nc.sync.dma_start(
                    xout[ht * 128:(ht + 1) * 128, :],
                    x_sb[:, ht * TOK:(ht + 1) * TOK],
                )
    return nc


def _get_nc():
    global _CACHED_NC
    if _CACHED_NC is None:
        _CACHED_NC = _build_nc()
    return _CACHED_NC


def _host_prep(inputs):
    """Fold ln/scale into weights, pre-transpose shards, embed gather, rope
    tables.  Returns in_maps (list of dicts, one per core)."""
    ids = np.asarray(inputs["input_ids"])[0]          # [S] int32
    embed = np.asarray(inputs["embed"], np.float32)   # [V, H]
    x = embed[ids]                                    # [S, H]

    inv = 1.0 / (10000.0 ** (np.arange(0, D, 2, dtype=np.float32) / D))  # [32]
    freqs = np.arange(S, dtype=np.float32)[:, None] * inv[None, :]       # [S,32]
    cosT = np.cos(freqs).T.astype(np.float32)   # [32, S]
    sinT = np.sin(freqs).T.astype(np.float32)
    cosF = np.ascontiguousarray(np.tile(cosT, (2, 1))).astype(BF)
    sinF = np.ascontiguousarray(np.concatenate([-sinT, sinT], 0)).astype(BF)

    def bf(a):
        return np.ascontiguousarray(a).astype(BF)

    scale = np.float32(1.0 / np.sqrt(D))
    in_maps = []
    for c in range(NCORES):
        m = {
            "x0": np.ascontiguousarray(x[c * TOK:(c + 1) * TOK, :].T),
            "cosf": cosF,
            "sinf": sinF,
        }
        for l in range(L):
            ln1 = np.asarray(inputs["ln1"], np.float32)[l]
            ln2 = np.asarray(inputs["ln2"], np.float32)[l]
            wq = np.asarray(inputs["Wq"], np.float32)[l] * ln1[None, :] * scale
            wk = np.asarray(inputs["Wk"], np.float32)[l] * ln1[None, :]
            wv = np.asarray(inputs["Wv"], np.float32)[l] * ln1[None, :]
            wo = np.asarray(inputs["Wo"], np.float32)[l]
            wg = np.asarray(inputs["Wg"], np.float32)[l] * ln2[None, :]
            wu = np.asarray(inputs["Wu"], np.float32)[l] * ln2[None, :]
            wd = np.asarray(inputs["Wd"], np.float32)[l]
            m[f"wq{l}"] = bf(wq[c * QD:(c + 1) * QD, :].T)
            m[f"wk{l}"] = bf(wk.T)
            m[f"wv{l}"] = bf(wv.T)
            m[f"wo{l}"] = bf(wo[:, c * QD:(c + 1) * QD].T)
            m[f"wg{l}"] = bf(wg[c * FFL:(c + 1) * FFL, :].T)
            m[f"wu{l}"] = bf(wu[c * FFL:(c + 1) * FFL, :].T)
            m[f"wd{l}"] = bf(wd[:, c * FFL:(c + 1) * FFL].T)
        in_maps.append(m)
    return in_maps


def kernel(**inputs) -> np.ndarray:
    nc = _get_nc()
    in_maps = _host_prep(inputs)
    res = bass_utils.run_bass_kernel_spmd(
        nc, in_maps, core_ids=list(range(NCORES))
    )
    out = np.empty((1, S, H), np.float32)
    for c in range(NCORES):
        out[0, c * TOK:(c + 1) * TOK, :] = res.results[c]["xout"].T
    return out


# revision 33
# speedup vs baseline: 1.4090x; 1.0131x over previous
"""Trainium2 Bass kernel for nn_CustomLlamaModel (2-layer MQA llama, B=1 S=2048
H=1024 HQ=16 HKV=1 FF=4096), tensor-parallel over 8 NeuronCores.

Strategy (per sharding hint): column-parallel q/gate/up, row-parallel o/down,
KV head replicated; residual stream token-sharded (256 tokens/core) and kept
feature-major [H, tok] fp32 in SBUF.  Per layer: rmsnorm (PE ones-matmul
partition reduce, fp32r) -> AllGather(h, bf16) -> qkv+rope -> transposed-
scores attention (softmax along partitions via exp + ones-column folded into
the attn@v matmul) -> row-parallel o-proj -> ReduceScatter(bf16) -> residual
add -> rmsnorm -> AllGather -> gate/up/silu -> row-parallel down ->
ReduceScatter -> residual add.  Matmul pipeline runs in bf16 (fp32 PSUM
accumulate); the gathered h stays fully SBUF-resident.  ln1/ln2 and the
1/sqrt(D) attention scale are folded into weights host-side; the embedding
gather runs host-side (pure numpy indexing).
"""
import sys

sys.path.insert(0, "/opt/trn_rl_repo")

import ml_dtypes
import numpy as np
import orjson

import concourse.bass as bass
import concourse.mybir as mybir
import concourse.tile as tile
from concourse import bass_utils
from concourse.masks import make_identity

# ---------------------------------------------------------------------------
# Walrus in this container supports only ONE sync-wait per instruction, but
# Tile's scheduler emits multi-wait instructions.  Post-process the BIR JSON:
# split each multi-wait instruction into single-wait NoOps (same engine,
# program-order before the original).
# ---------------------------------------------------------------------------
_orig_to_json_bytes = bass.Bass.to_json_bytes
_MW = [0]


def _split_multiwait(d):
    changed = False

    def fix_block(bb):
        nonlocal changed
        insts = bb.get("instructions")
        if not insts:
            return
        out = []
        for ins in insts:
            si = ins.get("sync_info")
            if si:
                ow = si.get("on_wait") or []
                if len(ow) > 1:
                    changed = True
                    for w in ow[:-1]:
                        _MW[0] += 1
                        out.append({
                            "debug": ins.get("debug", 0),
                            "engine": ins["engine"],
                            "ins": [],
                            "outs": [],
                            "name": f"{ins['name']}-mw{_MW[0]}",
                            "opcode": "NoOp",
                            "sync_info": {"on_update": [], "on_wait": [w]},
                        })
                    si["on_wait"] = [ow[-1]]
            out.append(ins)
        bb["instructions"] = out

    def rec(o):
        if isinstance(o, dict):
            if isinstance(o.get("instructions"), list):
                fix_block(o)
            for v in o.values():
                rec(v)
        elif isinstance(o, list):
            for v in o:
                rec(v)

    for fn in d.get("functions", []):
        rec(fn)
    return changed


def _patched_to_json_bytes(self):
    raw = _orig_to_json_bytes(self)
    d = orjson.loads(raw)
    if _split_multiwait(d):
        return orjson.dumps(d)
    return raw


bass.Bass.to_json_bytes = _patched_to_json_bytes

# ---------------------------------------------------------------------------
# Model / sharding constants
# ---------------------------------------------------------------------------
S, H, D, HQ, FF, L, V = 2048, 1024, 64, 16, 4096, 2, 32000
EPS = 1e-6
NCORES = 8
TOK = S // NCORES       # residual tokens per core (256)
QH = HQ // NCORES       # heads per core (2)
QD = QH * D             # q dims per core (128)
FFL = FF // NCORES      # ff dims per core (512)
NCH = 4                 # token chunks of 512
CH = S // NCH           # 512
KT = S // 128           # 16 key-token tiles
HT = H // 128           # 8 hidden feature tiles
F32 = mybir.dt.float32
F32R = mybir.dt.float32r
BF16 = mybir.dt.bfloat16
MUL = mybir.AluOpType.mult
ADD = mybir.AluOpType.add
RG = [list(range(NCORES))]
BF = ml_dtypes.bfloat16

_CACHED_NC = None


def _build_nc():
    nc = bass.Bass()
    x0 = nc.dram_tensor("x0", [H, TOK], F32, kind="ExternalInput")
    cosf = nc.dram_tensor("cosf", [64, S], BF16, kind="ExternalInput")
    sinf = nc.dram_tensor("sinf", [64, S], BF16, kind="ExternalInput")
    W = []
    for l in range(L):
        W.append({
            "wq": nc.dram_tensor(f"wq{l}", [H, QD], BF16, kind="ExternalInput"),
            "wk": nc.dram_tensor(f"wk{l}", [H, D], BF16, kind="ExternalInput"),
            "wv": nc.dram_tensor(f"wv{l}", [H, D], BF16, kind="ExternalInput"),
            "wo": nc.dram_tensor(f"wo{l}", [QD, H], BF16, kind="ExternalInput"),
            "wg": nc.dram_tensor(f"wg{l}", [H, FFL], BF16, kind="ExternalInput"),
            "wu": nc.dram_tensor(f"wu{l}", [H, FFL], BF16, kind="ExternalInput"),
            "wd": nc.dram_tensor(f"wd{l}", [FFL, H], BF16, kind="ExternalInput"),
        })
    xout = nc.dram_tensor("xout", [H, TOK], F32, kind="ExternalOutput")

    with tile.TileContext(nc) as tc:
        with (
            tc.tile_pool(name="const", bufs=1) as pconst,
            tc.tile_pool(name="resid", bufs=1) as presid,
            tc.tile_pool(name="wts", bufs=1) as pwts,
            tc.tile_pool(name="hfull", bufs=1) as phf,
            tc.tile_pool(name="acts", bufs=1) as pacts,
            tc.tile_pool(name="big", bufs=1) as pbig,
            tc.tile_pool(name="small", bufs=2) as psmall,
            tc.tile_pool(name="exp", bufs=3) as pexp,
            tc.tile_pool(name="outw", bufs=4) as poutw,
            tc.tile_pool(name="dram", bufs=2, space="DRAM") as pdram,
        ):
            # constants
            identf = pconst.tile([128, 128], F32, tag="identf")
            make_identity(nc, identf[:])
            ident = pconst.tile([128, 128], BF16, tag="ident")
            nc.vector.tensor_copy(ident[:], identf[:])
            onesf = pconst.tile([128, 128], F32, tag="onesf")
            nc.vector.memset(onesf[:], 1.0)
            ones = pconst.tile([128, 128], BF16, tag="ones")
            nc.vector.tensor_copy(ones[:], onesf[:])
            onesr = pconst.tile([128, 128], F32R, tag="onesr")
            nc.vector.tensor_copy(onesr[:], onesf[:])
            epst = pconst.tile([128, 1], F32, tag="eps")
            nc.gpsimd.memset(epst[:], EPS)
            cos_sb = pconst.tile([64, S], BF16, tag="cos")
            sin_sb = pconst.tile([64, S], BF16, tag="sin")
            nc.sync.dma_start(cos_sb[:], cosf[:])
            nc.sync.dma_start(sin_sb[:], sinf[:])

            # warmup: absorb the first-collective setup cost while the
            # x0 load and first norm run
            wrm_i = pdram.tile([128, 16], BF16, tag="warm_i")
            wrm_o = pdram.tile([NCORES * 128, 16], BF16, tag="warm_o",
                               addr_space="Shared")
            nc.gpsimd.collective_compute(
                "AllGather", mybir.AluOpType.bypass, replica_groups=RG,
                ins=[wrm_i[:].opt()], outs=[wrm_o[:].opt()],
            )

            # residual x, feature-major: block ht -> cols [ht*TOK:(ht+1)*TOK]
            x_sb = presid.tile([128, HT * TOK], F32, tag="x")
            for ht in range(HT):
                nc.sync.dma_start(
                    x_sb[:, ht * TOK:(ht + 1) * TOK],
                    x0[ht * 128:(ht + 1) * 128, :],
                )

            def rmsnorm_ag_load(tag):
                """x_sb -> normalized h (bf16) -> bounce -> AllGather (two
                feature-half chunks, pipelined) -> load into a fully-resident
                SBUF tile h_full [128, HT*S] (block k = features k*128..,
                cols = tokens in order)."""
                with tc.tile_pool(name=f"ps_n_{tag}", bufs=2,
                                  space="PSUM") as pps:
                    x2 = pbig.tile([128, HT * TOK], F32R, tag="rot")
                    ssq = pps.tile([1, TOK], F32, tag="ssq")
                    for ht in range(HT):
                        hs = slice(ht * TOK, (ht + 1) * TOK)
                        nc.vector.tensor_tensor(x2[:, hs], x_sb[:, hs],
                                                x_sb[:, hs], op=MUL)
                        nc.tensor.matmul(
                            ssq[:],
                            onesr[:, 0:1],
                            x2[:, ht * TOK:(ht + 1) * TOK],
                            start=(ht == 0),
                            stop=(ht == HT - 1),
                        )
                    sstd = psmall.tile([1, TOK], F32, tag="sstd")
                    nc.scalar.activation(
                        sstd[:], ssq[:], mybir.ActivationFunctionType.Sqrt,
                        bias=epst[0:1, :], scale=1.0 / H,
                    )
                    rinv = psmall.tile([1, TOK], F32R, tag="rinv")
                    with nc.allow_low_precision(reason="f32r is fp32 bits"):
                        nc.vector.reciprocal(rinv[:], sstd[:])
                    rb = pps.tile([128, TOK], F32, tag="rb")
                    nc.tensor.matmul(rb[:], onesr[0:1, :], rinv[:],
                                     start=True, stop=True)
                    h_sb = pbig.tile([128, HT * TOK], BF16, tag="h")
                    h_full = phf.tile([128, HT * S], BF16, tag="h_full")
                    HH = HT  # single chunk (collective floors dominate)
                    for j in range(1):
                        for ht in range(j * HH, (j + 1) * HH):
                            nc.vector.tensor_tensor(
                                h_sb[:, ht * TOK:(ht + 1) * TOK],
                                x_sb[:, ht * TOK:(ht + 1) * TOK],
                                rb[:],
                                op=MUL,
                            )
                        ag_in = pdram.tile([HH * 128, TOK], BF16,
                                           tag=f"ag_in{j}")
                        for ht in range(HH):
                            nc.sync.dma_start(
                                ag_in[ht * 128:(ht + 1) * 128, :],
                                h_sb[:, (j * HH + ht) * TOK:
                                     (j * HH + ht + 1) * TOK],
                            )
                        ag_out = pdram.tile([NCORES * HH * 128, TOK], BF16,
                                            tag=f"ag_out{j}",
                                            addr_space="Shared")
                        nc.gpsimd.collective_compute(
                            "AllGather",
                            mybir.AluOpType.bypass,
                            replica_groups=RG,
                            ins=[ag_in[:].opt()],
                            outs=[ag_out[:].opt()],
                        )
                        for k in range(HH):
                            for s in range(NCORES):
                                nc.sync.dma_start(
                                    h_full[:, (j * HH + k) * S + s * TOK:
                                           (j * HH + k) * S + (s + 1) * TOK],
                                    ag_out[s * HH * 128 + k * 128:
                                           s * HH * 128 + (k + 1) * 128, :],
                                )
                return h_full

            def reduce_scatter(o_parts):
                """o_parts: callable(m, n) -> sbuf bf16 tile [128, CH] holding
                the partial output for features m*128.., tokens n*CH..
                Feature-chunked (2 halves): RS of chunk 0 flies while the
                m=4..7 projections still run.  Adds results into x_sb."""
                HH = HT
                for j in range(1):
                    rs_in = pdram.tile([NCORES * HH * 128, TOK], BF16,
                                       tag=f"rs_in{j}")
                    for m in range(j * HH, (j + 1) * HH):
                        for n in range(NCH):
                            pt = o_parts(m, n)
                            for jj in range(CH // TOK):
                                s = (n * CH) // TOK + jj
                                nc.sync.dma_start(
                                    rs_in[s * HH * 128 + (m - j * HH) * 128:
                                          s * HH * 128 + (m - j * HH + 1) * 128,
                                          :],
                                    pt[:, jj * TOK:(jj + 1) * TOK],
                                )
                    rs_out = pdram.tile([HH * 128, TOK], BF16, tag=f"rs_out{j}")
                    nc.gpsimd.collective_compute(
                        "ReduceScatter",
                        mybir.AluOpType.add,
                        replica_groups=RG,
                        ins=[rs_in[:].opt()],
                        outs=[rs_out[:].opt()],
                    )
                    for ht in range(HH):
                        radd = poutw.tile([128, TOK], BF16, tag="radd")
                        nc.sync.dma_start(
                            radd[:], rs_out[ht * 128:(ht + 1) * 128, :]
                        )
                        nc.vector.tensor_tensor(
                            x_sb[:, (j * HH + ht) * TOK:
                                 (j * HH + ht + 1) * TOK],
                            x_sb[:, (j * HH + ht) * TOK:
                                 (j * HH + ht + 1) * TOK],
                            radd[:],
                            op=ADD,
                        )

            def rope(src, c0):
                """In-place RoPE on an S-column window of src starting at
                column c0 (feature-major [64, .] head block; cos/sin tables
                carry the sign pattern)."""
                cs = slice(c0, c0 + S)
                rot = pbig.tile([64, S], BF16, tag="ropescratch")
                nc.vector.tensor_copy(rot[0:32, :], src[32:64, cs])
                nc.vector.tensor_copy(rot[32:64, :], src[0:32, cs])
                nc.vector.tensor_tensor(
                    rot[:], rot[:], sin_sb[:, 0:S], op=MUL,
                )
                nc.vector.tensor_tensor(
                    src[:, cs], src[:, cs], cos_sb[:, 0:S], op=MUL,
                )
                nc.vector.tensor_tensor(
                    src[:, cs], src[:, cs], rot[:], op=ADD,
                )

            for l in range(L):
                w = W[l]
                # ---------------- attention ----------------
                h1 = rmsnorm_ag_load(f"a{l}")

                # weight loads (lhsT layouts)
                wq_sb = pwts.tile([128, HT * QD], BF16, tag="wq")
                wk_sb = pwts.tile([128, HT * D], BF16, tag="wk")
                wv_sb = pwts.tile([128, HT * D], BF16, tag="wv")
                wo_sb = pwts.tile([128, HT * 128], BF16, tag="wo")
                for k in range(HT):
                    nc.sync.dma_start(
                        wq_sb[:, k * QD:(k + 1) * QD],
                        w["wq"][k * 128:(k + 1) * 128, :],
                    )
                    nc.sync.dma_start(
                        wk_sb[:, k * D:(k + 1) * D],
                        w["wk"][k * 128:(k + 1) * 128, :],
                    )
                    nc.sync.dma_start(
                        wv_sb[:, k * D:(k + 1) * D],
                        w["wv"][k * 128:(k + 1) * 128, :],
                    )
                    nc.sync.dma_start(
                        wo_sb[:, k * 128:(k + 1) * 128],
                        w["wo"][:, k * 128:(k + 1) * 128],
                    )

                # q head-blocked [64, 2S]: head hh lives at cols hh*S..
                q_hb = pbig.tile([64, 2 * S], BF16, tag="q_hb")
                kTt = pbig.tile([64, S], BF16, tag="kT")
                vT = pbig.tile([64, S], BF16, tag="vT")
                with tc.tile_pool(name=f"ps_qkv{l}", bufs=2,
                                  space="PSUM") as pps:
                    for n in range(NCH):
                        pq = pps.tile([128, CH], F32, tag="pq")
                        pk = pps.tile([64, CH], F32, tag="pk")
                        pv = pps.tile([64, CH], F32, tag="pv")
                        for k in range(HT):
                            rhs = h1[:, k * S + n * CH: k * S + (n + 1) * CH]
                            nc.tensor.matmul(
                                pq[:], wq_sb[:, k * QD:(k + 1) * QD], rhs,
                                start=(k == 0), stop=(k == HT - 1),
                            )
                            nc.tensor.matmul(
                                pk[:], wk_sb[:, k * D:(k + 1) * D], rhs,
                                start=(k == 0), stop=(k == HT - 1),
                            )
                            nc.tensor.matmul(
                                pv[:], wv_sb[:, k * D:(k + 1) * D], rhs,
                                start=(k == 0), stop=(k == HT - 1),
                            )
                        ns = slice(n * CH, (n + 1) * CH)
                        nc.vector.tensor_copy(q_hb[:, ns], pq[0:64, :])
                        nc.vector.tensor_copy(
                            q_hb[:, S + n * CH:S + (n + 1) * CH],
                            pq[64:128, :],
                        )
                        nc.vector.tensor_copy(kTt[:, ns], pk[:])
                        nc.vector.tensor_copy(vT[:, ns], pv[:])

                    rope(q_hb, 0)
                    rope(q_hb, S)
                    rope(kTt, 0)

                    # token-major v with ones column (softmax sum rides attn@v)
                    v_tok = pbig.tile([128, KT * 65], BF16, tag="v_tok")
                    for kt in range(KT):
                        nc.vector.tensor_copy(
                            v_tok[:, kt * 65 + 64:kt * 65 + 65],
                            ones[:, 0:1],
                        )
                    for kt in range(KT):
                        pvt = pps.tile([128, 64], BF16, tag="pvt")
                        nc.tensor.transpose(
                            pvt[:], vT[:, kt * 128:(kt + 1) * 128],
                            ident[0:64, 0:64],
                        )
                        nc.vector.tensor_copy(
                            v_tok[:, kt * 65:kt * 65 + 64], pvt[:]
                        )

                oT = pbig.tile([128, S], BF16, tag="oT")
                with tc.tile_pool(name=f"ps_att{l}", bufs=1,
                                  space="PSUM") as ppa:
                    for hh in range(QH):
                        for qh in range(2):
                            q0 = qh * (S // 2)        # q-column base
                            pav = ppa.tile([65, S // 2], F32, tag="pav",
                                           bufs=1)
                            for kt in range(KT):
                                psc = ppa.tile([128, 2 * CH], F32, tag="psc",
                                               bufs=3)
                                for j in range(2):
                                    nc.tensor.matmul(
                                        psc[:, j * CH:(j + 1) * CH],
                                        kTt[:, kt * 128:(kt + 1) * 128],
                                        q_hb[:, hh * S + q0 + j * CH:
                                             hh * S + q0 + (j + 1) * CH],
                                        start=True, stop=True,
                                    )
                                et = pexp.tile([128, 2 * CH], BF16, tag="et")
                                nc.scalar.activation(
                                    et[:], psc[:],
                                    mybir.ActivationFunctionType.Exp,
                                )
                                for j in range(2):
                                    nc.tensor.matmul(
                                        pav[:, j * CH:(j + 1) * CH],
                                        v_tok[:, kt * 65:(kt + 1) * 65],
                                        et[:, j * CH:(j + 1) * CH],
                                        start=(kt == 0), stop=(kt == KT - 1),
                                        skip_group_check=True,
                                    )
                            for j in range(2):
                                js = slice(j * CH, (j + 1) * CH)
                                rec = psmall.tile([1, CH], F32R, tag="rec")
                                with nc.allow_low_precision(
                                    reason="f32r is fp32 bits"
                                ):
                                    nc.vector.reciprocal(rec[:],
                                                         pav[64:65, js])
                                rbc = ppa.tile([64, CH], F32, tag="psc",
                                               bufs=3)
                                nc.tensor.matmul(
                                    rbc[:], onesr[0:1, 0:64], rec[:],
                                    start=True, stop=True,
                                )
                                rbs = poutw.tile([64, CH], F32, tag="rbs")
                                nc.vector.tensor_copy(rbs[:], rbc[:])
                                nc.vector.tensor_tensor(
                                    oT[hh * 64:(hh + 1) * 64,
                                       q0 + j * CH:q0 + (j + 1) * CH],
                                    pav[0:64, js], rbs[:], op=MUL,
                                )

                # row-parallel o-proj -> RS -> residual add
                with tc.tile_pool(name=f"ps_o{l}", bufs=4,
                                  space="PSUM") as ppo:
                    def o_part(m, n, _oT=oT, _wo=wo_sb, _pp=ppo):
                        po = _pp.tile([128, CH], F32, tag="po")
                        nc.tensor.matmul(
                            po[:], _wo[:, m * 128:(m + 1) * 128],
                            _oT[:, n * CH:(n + 1) * CH],
                            start=True, stop=True,
                        )
                        ot = poutw.tile([128, CH], BF16, tag="opart")
                        nc.vector.tensor_copy(ot[:], po[:])
                        return ot

                    reduce_scatter(o_part)

                # ---------------- mlp ----------------
                h2 = rmsnorm_ag_load(f"m{l}")
                wg_sb = pwts.tile([128, HT * FFL], BF16, tag="wg")
                wu_sb = pwts.tile([128, HT * FFL], BF16, tag="wu")
                wd_sb = pwts.tile([128, 4 * H], BF16, tag="wd")
                for k in range(HT):
                    nc.sync.dma_start(
                        wg_sb[:, k * FFL:(k + 1) * FFL],
                        w["wg"][k * 128:(k + 1) * 128, :],
                    )
                    nc.sync.dma_start(
                        wu_sb[:, k * FFL:(k + 1) * FFL],
                        w["wu"][k * 128:(k + 1) * 128, :],
                    )
                for fk in range(4):
                    nc.sync.dma_start(
                        wd_sb[:, fk * H:(fk + 1) * H],
                        w["wd"][fk * 128:(fk + 1) * 128, :],
                    )

                act = pacts.tile([128, 4 * S], BF16, tag="act")
                with tc.tile_pool(name=f"ps_gu{l}", bufs=2,
                                  space="PSUM") as ppg:
                    for f in range(4):
                        for n in range(NCH):
                            pg = ppg.tile([128, CH], F32, tag="pg")
                            pu = ppg.tile([128, CH], F32, tag="pu")
                            for k in range(HT):
                                rhs = h2[:, k * S + n * CH:
                                         k * S + (n + 1) * CH]
                                nc.tensor.matmul(
                                    pg[:],
                                    wg_sb[:, k * FFL + f * 128:
                                          k * FFL + (f + 1) * 128],
                                    rhs,
                                    start=(k == 0), stop=(k == HT - 1),
                                )
                                nc.tensor.matmul(
                                    pu[:],
                                    wu_sb[:, k * FFL + f * 128:
                                          k * FFL + (f + 1) * 128],
                                    rhs,
                                    start=(k == 0), stop=(k == HT - 1),
                                )
                            ns = slice(f * S + n * CH, f * S + (n + 1) * CH)
                            nc.scalar.activation(
                                act[:, ns], pg[:],
                                mybir.ActivationFunctionType.Silu,
                            )
                            ut = poutw.tile([128, CH], BF16, tag="ut")
                            nc.vector.tensor_copy(ut[:], pu[:])
                            nc.vector.tensor_tensor(
                                act[:, ns], act[:, ns], ut[:], op=MUL,
                            )

                with tc.tile_pool(name=f"ps_d{l}", bufs=4,
                                  space="PSUM") as ppd:
                    def d_part(m, n, _act=act, _wd=wd_sb, _pp=ppd):
                        pd = _pp.tile([128, CH], F32, tag="pd")
                        for fk in range(4):
                            nc.tensor.matmul(
                                pd[:],
                                _wd[:, fk * H + m * 128:
                                    fk * H + (m + 1) * 128],
                                _act[:, fk * S + n * CH:
                                     fk * S + (n + 1) * CH],
                                start=(fk == 0), stop=(fk == 3),
                            )
                        ot = poutw.tile([128, CH], BF16, tag="opart")
                        nc.vector.tensor_copy(ot[:], pd[:])
                        return ot

                    reduce_scatter(d_part)

            # final output
            for ht in range(HT):
                nc.sync.dma_start(
                    xout[ht * 128:(ht + 1) * 128, :],
                    x_sb[:, ht * TOK:(ht + 1) * TOK],
                )
    return nc


def _get_nc():
    global _CACHED_NC
    if _CACHED_NC is None:
        _CACHED_NC = _build_nc()
    return _CACHED_NC


def _host_prep(inputs):
    """Fold ln/scale into weights, pre-transpose shards, embed gather, rope
    tables.  Returns in_maps (list of dicts, one per core)."""
    ids = np.asarray(inputs["input_ids"])[0]          # [S] int32
    embed = np.asarray(inputs["embed"], np.float32)   # [V, H]
    x = embed[ids]                                    # [S, H]

    inv = 1.0 / (10000.0 ** (np.arange(0, D, 2, dtype=np.float32) / D))  # [32]
    freqs = np.arange(S, dtype=np.float32)[:, None] * inv[None, :]       # [S,32]
    cosT = np.cos(freqs).T.astype(np.float32)   # [32, S]
    sinT = np.sin(freqs).T.astype(np.float32)
    cosF = np.ascontiguousarray(np.tile(cosT, (2, 1))).astype(BF)
    sinF = np.ascontiguousarray(np.concatenate([-sinT, sinT], 0)).astype(BF)

    def bf(a):
        return np.ascontiguousarray(a).astype(BF)

    scale = np.float32(1.0 / np.sqrt(D))
    in_maps = []
    for c in range(NCORES):
        m = {
            "x0": np.ascontiguousarray(x[c * TOK:(c + 1) * TOK, :].T),
            "cosf": cosF,
            "sinf": sinF,
        }
        for l in range(L):
            ln1 = np.asarray(inputs["ln1"], np.float32)[l]
            ln2 = np.asarray(inputs["ln2"], np.float32)[l]
            wq = np.asarray(inputs["Wq"], np.float32)[l] * ln1[None, :] * scale
            wk = np.asarray(inputs["Wk"], np.float32)[l] * ln1[None, :]
            wv = np.asarray(inputs["Wv"], np.float32)[l] * ln1[None, :]
            wo = np.asarray(inputs["Wo"], np.float32)[l]
            wg = np.asarray(inputs["Wg"], np.float32)[l] * ln2[None, :]
            wu = np.asarray(inputs["Wu"], np.float32)[l] * ln2[None, :]
            wd = np.asarray(inputs["Wd"], np.float32)[l]
            m[f"wq{l}"] = bf(wq[c * QD:(c + 1) * QD, :].T)
            m[f"wk{l}"] = bf(wk.T)
            m[f"wv{l}"] = bf(wv.T)
            m[f"wo{l}"] = bf(wo[:, c * QD:(c + 1) * QD].T)
            m[f"wg{l}"] = bf(wg[c * FFL:(c + 1) * FFL, :].T)
            m[f"wu{l}"] = bf(wu[c * FFL:(c + 1) * FFL, :].T)
            m[f"wd{l}"] = bf(wd[:, c * FFL:(c + 1) * FFL].T)
        in_maps.append(m)
    return in_maps


def kernel(**inputs) -> np.ndarray:
    nc = _get_nc()
    in_maps = _host_prep(inputs)
    res = bass_utils.run_bass_kernel_spmd(
        nc, in_maps, core_ids=list(range(NCORES))
    )
    out = np.empty((1, S, H), np.float32)
    for c in range(NCORES):
        out[0, c * TOK:(c + 1) * TOK, :] = res.results[c]["xout"].T
    return out
